# revision 24
# baseline (speedup 1.0000x reference)
"""Self-contained TRN2 Bass kernel for nn_EnhancedMultiheadAttention.

kernel(**inputs) takes the FULL unsharded inputs (x, Wq, bq, Wk, bk, Wv, bv,
Wo, bo as float32 numpy arrays), distributes the computation across 8
NeuronCores (tensor-parallel over heads: core c owns heads 2c, 2c+1), and
returns the full [2, 2048, 1024] float32 output.

v2 design notes:
- all matmul operands in bf16 (fast weight load, half DMA/SBUF traffic);
  accumulation stays fp32 in PSUM.
- QK^T packs both local heads into one PE pass via 64-row tile_position
  row-tiles running concurrently (heads stacked on partitions 0:64/64:128).
- V projection is computed directly in [token, dim] layout (lhsT=x chunk)
  so no PE transposes are needed; a ones-column rides along for the
  softmax denominator.
- output tokens are sharded interleaved per 1024-token block (core c owns
  tokens blk*1024 + c*128 ..+128) so one small bf16 AllToAll fires per
  block and overlaps the remaining attention; output projection runs in
  the tail, overlapping the last A2A.
- softmax denominator reciprocal uses reciprocal_approx_fast + a DRAM
  bounce row for the partition broadcast.
"""

import sys

for _p in ("/opt/trn_rl_repo", "/root/.axon_site/_ro/trn_rl_repo"):
    if _p not in sys.path:
        sys.path.append(_p)


import numpy as np

import concourse.bass as bass
import concourse.mybir as mybir
import concourse.tile as tile
import bass_rust

F32 = mybir.dt.float32
B16 = mybir.dt.bfloat16

B, L, D = 2, 2048, 1024
H, DH = 16, 64
NCORES = 8
T = B * L                  # 4096 flattened tokens
NKC = D // 128             # 8 contraction chunks of 128
CHUNK = 1024               # projection token-chunk width
IB = 1024                  # query block width
NI = L // IB               # 2 query blocks per batch
NJ = L // 128              # 16 key chunks of 128 per batch
HPC = H // NCORES          # 2 heads per core
NBLK = T // IB             # 4 token blocks (A2A granularity)
TPC_BLK = IB // NCORES     # 128 tokens per core per block
TC = NBLK * TPC_BLK        # 512 output tokens per core


def split_excess_waits(nc, max_waits=1):
    """walrus's setupSyncWait rejects instructions with more than one wait
    condition on this compiler version; hoist extras onto preceding NoOps."""
    n_split = 0
    for f in nc.m.functions:
        for b in f.blocks:
            new_list = None
            for inst in list(b.instructions):
                si = inst.sync_info
                if si is None or len(si.on_wait) <= max_waits:
                    continue
                waits = list(si.on_wait)
                keep = waits[-max_waits:]
                excess = waits[:-max_waits]
                nops = []
                for j, w in enumerate(excess):
                    nop = mybir.InstNoOp(
                        name=f"I-wsplit-{inst.name}-{j}", ins=[], outs=[],
                        engine=inst.engine,
                    )
                    nop.sync_info = bass_rust.SyncInfo(on_wait=[w], on_update=[])
                    nops.append(nop)
                inst.sync_info = bass_rust.SyncInfo(
                    on_wait=keep, on_update=list(si.on_update)
                )
                if new_list is None:
                    new_list = list(b.instructions)
                pos = new_list.index(inst)
                new_list[pos:pos] = nops
                n_split += 1
            if new_list is not None:
                b.instructions = new_list
    return n_split


def build_nc(proj_mm=None, attn_mm=None, out_mm=None):
    nc = bass.Bass("TRN2", target_bir_lowering=False, debug=False,
                   num_devices=NCORES)

    xT = nc.dram_tensor("xT", [D, T], B16, kind="ExternalInput").ap()
    wq = nc.dram_tensor("wq", [D, 128], B16, kind="ExternalInput").ap()
    wk = nc.dram_tensor("wk", [D, 128], B16, kind="ExternalInput").ap()
    wv = nc.dram_tensor("wv", [D, 128], B16, kind="ExternalInput").ap()
    bq = nc.dram_tensor("bq", [128, 1], F32, kind="ExternalInput").ap()
    bk = nc.dram_tensor("bk", [128, 1], F32, kind="ExternalInput").ap()
    bv = nc.dram_tensor("bv", [128, 1], F32, kind="ExternalInput").ap()
    wo = nc.dram_tensor("wo", [D, D], B16, kind="ExternalInput").ap()
    bo = nc.dram_tensor("bo", [1, D], F32, kind="ExternalInput").ap()
    cosT = nc.dram_tensor("cosT", [128, L], B16, kind="ExternalInput").ap()
    sinT = nc.dram_tensor("sinT", [128, L], B16, kind="ExternalInput").ap()
    out = nc.dram_tensor("out", [TC, D], F32, kind="ExternalOutput").ap()

    with tile.TileContext(nc) as tc:
        _build_body(nc, tc, xT, wq, wk, wv, bq, bk, bv, wo, bo, cosT, sinT,
                    out)

    split_excess_waits(nc)
    return nc


def _build_body(nc, tc, xT, wq, wk, wv, bq, bk, bv, wo, bo, cosT, sinT, out):
    from contextlib import ExitStack

    ctx = ExitStack()
    with ctx:
        # ---------------- persistent tensors ----------------
        persist = ctx.enter_context(tc.tile_pool(name="persist", bufs=1))
        wpool = ctx.enter_context(tc.tile_pool(name="wqkv", bufs=1))

        w_t = {}
        b_t = {}
        for name, wap, bap in (("q", wq, bq), ("k", wk, bk), ("v", wv, bv)):
            w_t[name] = wpool.tile([128, NKC, 128], B16, tag=f"w{name}",
                                   name=f"w{name}")
            nc.sync.dma_start(w_t[name][:],
                              wap.rearrange("(kc p) m -> p kc m", p=128))
            b_t[name] = wpool.tile([128, 1], F32, tag=f"b{name}",
                                   name=f"b{name}")
            nc.sync.dma_start(b_t[name][:], bap)

        cos_l = persist.tile([128, L], B16, tag="cos", name="cos")
        sin_l = persist.tile([128, L], B16, tag="sin", name="sin")
        nc.sync.dma_start(cos_l[:], cosT[:])
        nc.sync.dma_start(sin_l[:], sinT[:])

        ones_col = persist.tile([128, 1], F32, tag="ones", name="ones")
        nc.gpsimd.memset(ones_col[:], 1.0)

        ident = persist.tile([128, 128], B16, tag="ident", name="ident")
        from concourse.masks import make_identity
        make_identity(nc, ident[:])

        # per-batch Q/K (heads stacked on partitions) and V ([token, dim])
        qt_sb = [persist.tile([128, L], B16, tag=f"qt{b}", name=f"qt{b}")
                 for b in range(B)]
        kt_sb = [persist.tile([128, L], B16, tag=f"kt{b}", name=f"kt{b}")
                 for b in range(B)]
        v_sb = [[persist.tile([128, NJ, 66], B16, tag=f"v{b}{h}",
                              name=f"v{b}{h}")
                 for h in range(HPC)] for b in range(B)]
        for b in range(B):
            for h in range(HPC):
                nc.vector.tensor_copy(
                    v_sb[b][h][:, :, 64:65],
                    ones_col[:, :].to_broadcast((128, NJ, 1)))

        # context after A2A: [128 dims-of-kc, kc, blk, tok]
        opool = ctx.enter_context(tc.tile_pool(name="oproj", bufs=1))
        wo_sb = opool.tile([128, NKC, D], B16, tag="wo", name="wo")
        bo_sb = opool.tile([128, D], F32, tag="bo", name="bo")
        ctx_sb = opool.tile([128, NKC, NBLK, TPC_BLK], B16, tag="ctxsb",
                            name="ctxsb")

        # DRAM buffers for the per-block collectives (bf16)
        dram = ctx.enter_context(tc.tile_pool(name="dram", bufs=1,
                                              space="DRAM"))
        a2a_in = [dram.tile([NCORES * DH, HPC * TPC_BLK], B16,
                            name=f"a2a_in{blk}") for blk in range(NBLK)]
        a2a_out = [dram.tile([NCORES * DH, HPC * TPC_BLK], B16,
                             name=f"a2a_out{blk}") for blk in range(NBLK)]

        xT3 = xT.rearrange("(kc p) t -> p kc t", p=128)

        # prefetch ALL x chunks up front (sync queue) so the inter-batch
        # transition never waits on HBM
        xpool = ctx.enter_context(tc.tile_pool(name="x", bufs=4))
        xt_c = []
        for ci in range(T // CHUNK):
            xt = xpool.tile([128, NKC, CHUNK], B16, tag="xchunk",
                            name=f"xchunk{ci}")
            for kc in range(NKC):
                nc.sync.dma_start(xt[:, kc, :], xT3[:, kc, bass.ts(ci, CHUNK)])
            xt_c.append(xt)
        # big output-projection weights load overlaps everything
        nc.sync.dma_start(wo_sb[:],
                          wo.rearrange("(kc p) n -> p kc n", p=128))
        nc.sync.dma_start(bo_sb[:], bo.to_broadcast((128, D)))

        for b in range(B):
            # ---------------- phase A(b): projections + RoPE ----------------
            actx = ExitStack()
            tmp = actx.enter_context(tc.tile_pool(name="ptmp", bufs=3))
            ppsum = actx.enter_context(
                tc.tile_pool(name="ppsum", bufs=2, space="PSUM"))
            vtpsum = actx.enter_context(
                tc.tile_pool(name="vtpsum", bufs=2, space="PSUM"))

            for half in range(L // CHUNK):
                ci = b * (L // CHUNK) + half       # global chunk id
                lsl = bass.ts(half, CHUNK)         # slice into qt/kt/cos/sin
                xt = xt_c[ci]
                cos_t = cos_l[:, lsl]
                sin_t = sin_l[:, lsl]

                # Q/K/V projections: [dims, tokens]
                for name in ("q", "k", "v"):
                    ps = ppsum.tile([128, CHUNK], F32, tag="proj", name="proj")
                    for kc in range(NKC):
                        for nh in range(CHUNK // 512):
                            nc.tensor.matmul(
                                ps[:, bass.ts(nh, 512)],
                                w_t[name][:, kc, :],
                                xt[:, kc, bass.ts(nh, 512)],
                                start=(kc == 0), stop=(kc == NKC - 1),
                            )
                    if name == "v":
                        raw = tmp.tile([128, CHUNK], B16, tag="raw",
                                       name="raw")
                        nc.vector.tensor_scalar_add(raw[:], ps[:], b_t["v"][:])
                        # transpose [64, 128] blocks into [token, dim] via PE
                        for h in range(HPC):
                            o = h * DH
                            for sub in range(CHUNK // 128):
                                kb = half * (CHUNK // 128) + sub
                                tp = vtpsum.tile([128, DH], B16, tag="vt",
                                                 name="vt")
                                nc.tensor.transpose(
                                    tp[:],
                                    raw[bass.ds(o, DH), bass.ts(sub, 128)],
                                    ident[bass.ds(o, DH), bass.ds(o, DH)],
                                )
                                nc.vector.tensor_copy(
                                    v_sb[b][h][:, kb, 0:DH], tp[:])
                        continue
                    # Q/K: bias then RoPE
                    raw = tmp.tile([128, CHUNK], B16, tag="raw", name="raw")
                    nc.vector.tensor_scalar_add(raw[:], ps[:], b_t[name][:])
                    shifted = tmp.tile([128, CHUNK], B16, tag="shift",
                                       name="shift")
                    for h in range(HPC):
                        o = h * DH
                        nc.scalar.dma_start(shifted[o:o + 32, :],
                                            raw[o + 32:o + 64, :])
                        nc.scalar.dma_start(shifted[o + 32:o + 64, :],
                                            raw[o:o + 32, :])
                    t1 = tmp.tile([128, CHUNK], B16, tag="t1", name="t1")
                    nc.vector.tensor_mul(t1[:], raw[:], cos_t)
                    nc.vector.tensor_mul(shifted[:], shifted[:], sin_t)
                    dst = qt_sb[b] if name == "q" else kt_sb[b]
                    nc.vector.tensor_add(dst[:, lsl], t1[:], shifted[:])

            actx.close()

            # ---------------- phase B(b): attention ----------------
            bctx = ExitStack()
            ppool = bctx.enter_context(tc.tile_pool(name="pT", bufs=4))
            avspool = bctx.enter_context(tc.tile_pool(name="avs", bufs=2))
            rrpool = bctx.enter_context(tc.tile_pool(name="rr", bufs=2))
            rbpool = bctx.enter_context(tc.tile_pool(name="rb", bufs=2))
            cxpool = bctx.enter_context(tc.tile_pool(name="cx", bufs=2))
            stpsum = bctx.enter_context(
                tc.tile_pool(name="stpsum", bufs=2, space="PSUM"))
            avpsum = bctx.enter_context(
                tc.tile_pool(name="avpsum", bufs=2, space="PSUM"))
            ndram = bctx.enter_context(
                tc.tile_pool(name="ndram", bufs=2, space="DRAM"))

            for ib in range(NI):
                blk = b * NI + ib
                # av rows 0:65 hold the AV accumulation; rows 96:128 are a
                # scratch target for PE warm-keeper fillers (HAM drops the
                # PE clock to 1.2 GHz unless the array stays busy).
                av = [avpsum.tile([128, IB], F32, tag="av", name="av")
                      for _ in range(HPC)]
                pt_prev = None
                for jc in range(NJ):
                    st = [stpsum.tile([128, IB], F32, tag="st", name="st")
                          for _ in range(HPC)]
                    ksl = bass.ds(jc * 128, 128)
                    # QK^T for both heads concurrently (64-row PE tiles)
                    for nh in range(IB // 512):
                        qsl = bass.ds(ib * IB + nh * 512, 512)
                        for h in range(HPC):
                            o = h * DH
                            nc.tensor.matmul(
                                st[h][:, bass.ts(nh, 512)],
                                kt_sb[b][o:o + DH, ksl],
                                qt_sb[b][o:o + DH, qsl],
                                start=True, stop=True,
                            )
                    pt = [ppool.tile([128, IB], B16, tag="pt", name="pt")
                          for _ in range(HPC)]
                    for h in range(HPC):
                        nc.scalar.activation(pt[h][:], st[h][:],
                                             mybir.ActivationFunctionType.Exp,
                                             scale=float(DH) ** -0.5)
                    if pt_prev is not None:
                        _emit_av(nc, av, v_sb[b], pt_prev, jc - 1)
                        # PE warm-keeper: results never read, rows 96:128
                        for h in range(HPC):
                            nc.tensor.matmul(
                                av[h][96:128, 0:512],
                                ident[0:1, 0:32], qt_sb[b][0:1, 0:512],
                                start=False, stop=True,
                                skip_group_check=True,
                                tile_position=(0, 96),
                            )
                    pt_prev = pt
                _emit_av(nc, av, v_sb[b], pt_prev, NJ - 1)

                # normalize: den is row DH of av. Bounce it through DRAM
                # reshaped to [128, IB//128] so reciprocal runs on all 128
                # DVE lanes (the [1, IB] form is ~8 cyc/elem on one lane),
                # then bounce back and broadcast-read over DH partitions.
                for h in range(HPC):
                    avs = avspool.tile([DH + 1, IB], F32, tag="avs",
                                       name="avs")
                    nc.vector.tensor_copy(avs[:], av[h][0:DH + 1, :])
                    rr = rrpool.tile([DH + 1, IB], F32, tag="rr", name="rr")
                    nc.vector.reciprocal(rr[DH:DH + 1, :], avs[DH:DH + 1, :])
                    rrow = ndram.tile([1, IB], F32, tag="rrow", name="rrow")
                    nc.sync.dma_start(rrow[:], rr[DH:DH + 1, :])
                    rb = rbpool.tile([DH, IB], F32, tag="rb", name="rb")
                    nc.sync.dma_start(rb[:], rrow[0:1, :].to_broadcast((DH, IB)))
                    cx = cxpool.tile([DH, IB], B16, tag="cx", name="cx")
                    nc.vector.tensor_mul(cx[:], avs[0:DH, :], rb[:])
                    # stage into the A2A buffer: shard s gets this head's
                    # tokens s*128..+128 (cols h*128..+128 of the shard)
                    for s in range(NCORES):
                        nc.sync.dma_start(
                            a2a_in[blk][bass.ds(s * DH, DH),
                                        bass.ds(h * TPC_BLK, TPC_BLK)],
                            cx[:, bass.ts(s, TPC_BLK)])

                nc.gpsimd.collective_compute(
                    "AllToAll",
                    mybir.AluOpType.bypass,
                    replica_groups=[list(range(NCORES))],
                    ins=[a2a_in[blk][:]],
                    outs=[a2a_out[blk][:]],
                )
                for h in range(HPC):
                    src = a2a_out[blk][:, bass.ds(h * TPC_BLK, TPC_BLK)] \
                        .rearrange("(g p) t -> p g t", p=DH)
                    nc.gpsimd.dma_start(
                        ctx_sb[bass.ds(h * DH, DH), :, blk, :], src)

            bctx.close()

        # ---------------- phase C: output projection (tail) ----------------
        ostage = ctx.enter_context(tc.tile_pool(name="ostage", bufs=2))
        opsum = ctx.enter_context(tc.tile_pool(name="opsum", bufs=4,
                                               space="PSUM"))

        for blk in range(NBLK):
            pss = [opsum.tile([128, 512], F32, tag="ops", name=f"ops{blk}{nh}")
                   for nh in range(2)]
            for kc in range(NKC):
                for nh in range(2):
                    nc.tensor.matmul(
                        pss[nh][:],
                        ctx_sb[:, kc, blk, :],
                        wo_sb[:, kc, bass.ts(nh, 512)],
                        start=(kc == 0), stop=(kc == NKC - 1),
                    )
            for nh in range(2):
                ot = ostage.tile([128, 512], F32, tag="ot", name="ot")
                nc.vector.tensor_add(ot[:], pss[nh][:],
                                     bo_sb[:, bass.ts(nh, 512)])
                nc.sync.dma_start(
                    out[bass.ts(blk, TPC_BLK), bass.ts(nh, 512)], ot[:])


def _emit_av(nc, av, v_b, pt, jc):
    """AV accumulation for key chunk jc, both heads."""
    for h in range(HPC):
        for nh in range(IB // 512):
            nc.tensor.matmul(
                av[h][0:DH + 1, bass.ts(nh, 512)],
                v_b[h][:, jc, 0:DH + 1],
                pt[h][:, bass.ts(nh, 512)],
                start=(jc == 0), stop=(jc == NJ - 1),
            )


# ---------------- host-side sharding / unsharding ----------------

def _bf16(a):
    import ml_dtypes
    return np.ascontiguousarray(a).astype(ml_dtypes.bfloat16)


def rope_cos_sin_np(seq_len, d_head):
    inv_freq = 1.0 / (10000.0 ** (np.arange(0, d_head, 2, dtype=np.float32) / d_head))
    t = np.arange(seq_len, dtype=np.float32)
    freqs = np.einsum("i,j->ij", t, inv_freq).astype(np.float32)
    emb = np.concatenate((freqs, freqs), axis=-1)
    return np.cos(emb).astype(np.float32), np.sin(emb).astype(np.float32)


def make_in_maps(x, Wq, bq, Wk, bk, Wv, bv, Wo, bo):
    xT = _bf16(x.reshape(T, D).T)

    cos, sin = rope_cos_sin_np(L, DH)          # [L, 64]
    cosT = cos.T                               # [64, L]
    sinT = sin.T
    sgn = np.where(np.arange(DH) < DH // 2, -1.0, 1.0).astype(np.float32)
    sinT_signed = sinT * sgn[:, None]
    cosT_full = _bf16(np.tile(cosT, (HPC, 1)))      # [128, 2048]
    sinT_full = _bf16(np.tile(sinT_signed, (HPC, 1)))

    wo_full = _bf16(Wo)
    bo_row = np.ascontiguousarray(bo.reshape(1, D))

    in_maps = []
    for c in range(NCORES):
        sl = slice(c * 128, (c + 1) * 128)
        in_maps.append({
            "xT": xT,
            "wq": _bf16(Wq[:, sl]),
            "wk": _bf16(Wk[:, sl]),
            "wv": _bf16(Wv[:, sl]),
            "bq": np.ascontiguousarray(bq[sl].reshape(128, 1)),
            "bk": np.ascontiguousarray(bk[sl].reshape(128, 1)),
            "bv": np.ascontiguousarray(bv[sl].reshape(128, 1)),
            "wo": wo_full,
            "bo": bo_row,
            "cosT": cosT_full,
            "sinT": sinT_full,
        })
    return in_maps


def assemble_output(results):
    outs = np.stack([results[c]["out"] for c in range(NCORES)])  # [8, 512, D]
    full = outs.reshape(NCORES, NBLK, TPC_BLK, D).transpose(1, 0, 2, 3)
    return np.ascontiguousarray(full.reshape(B, L, D)).astype(np.float32)


_CACHE = {}


def _get_runner():
    """Build the Bass program and a cached jitted SPMD executor once.

    Mirrors bass2jax.run_bass_via_pjrt's multi-core path, but keeps the
    jitted shard_map callable alive so repeat kernel() calls skip retracing.
    """
    if "runner" in _CACHE:
        return _CACHE["runner"]

    import jax
    import numpy as _np
    from jax.sharding import Mesh, PartitionSpec
    from jax.experimental.shard_map import shard_map
    from concourse import bass2jax, mybir as _mybir

    nc = build_nc()
    bass2jax.install_neuronx_cc_hook()

    partition_name = (nc.partition_id_tensor.name
                      if nc.partition_id_tensor else None)
    in_names, out_names, out_avals, zero_shapes = [], [], [], []
    for alloc in nc.m.functions[0].allocations:
        if not isinstance(alloc, _mybir.MemoryLocationSet):
            continue
        name = alloc.memorylocations[0].name
        if alloc.kind == "ExternalInput":
            if name != partition_name:
                in_names.append(name)
        elif alloc.kind == "ExternalOutput":
            shape = tuple(alloc.tensor_shape)
            dtype = _mybir.dt.np(alloc.dtype)
            out_names.append(name)
            out_avals.append(jax.core.ShapedArray(shape, dtype))
            zero_shapes.append((shape, dtype))
    n_params = len(in_names)
    n_outs = len(out_avals)
    all_in_names = list(in_names) + list(out_names)
    if partition_name is not None:
        all_in_names.append(partition_name)
    donate = tuple(range(n_params, n_params + n_outs))

    def _body(*args):
        operands = list(args)
        if partition_name is not None:
            operands.append(bass2jax.partition_id_tensor())
        outs = bass2jax._bass_exec_p.bind(
            *operands,
            out_avals=tuple(out_avals),
            in_names=tuple(all_in_names),
            out_names=tuple(out_names),
            lowering_input_output_aliases=(),
            sim_require_finite=True,
            sim_require_nnan=True,
            nc=nc,
        )
        return tuple(outs)

    devices = jax.devices()[:NCORES]
    mesh = Mesh(_np.asarray(devices), ("core",))
    in_specs = (PartitionSpec("core"),) * (n_params + n_outs)
    out_specs = (PartitionSpec("core"),) * n_outs
    sharded = jax.jit(
        shard_map(_body, mesh=mesh, in_specs=in_specs, out_specs=out_specs,
                  check_rep=False),
        donate_argnums=donate,
        keep_unused=True,
    )

    def run(in_maps):
        per_core = [[_np.asarray(m[name]) for name in in_names]
                    for m in in_maps]
        concat_in = [
            _np.concatenate([per_core[c][i] for c in range(NCORES)], axis=0)
            for i in range(n_params)
        ]
        concat_zeros = [
            _np.zeros((NCORES * s[0], *s[1:]), dt) for s, dt in zero_shapes
        ]
        out_arrs = sharded(*concat_in, *concat_zeros)
        return [
            {name: _np.asarray(out_arrs[i]).reshape(
                NCORES, *out_avals[i].shape)[c]
             for i, name in enumerate(out_names)}
            for c in range(NCORES)
        ]

    _CACHE["runner"] = run
    return run


def kernel(**inputs):
    run = _get_runner()
    in_maps = make_in_maps(**{k: np.asarray(v, dtype=np.float32)
                              for k, v in inputs.items()})
    return assemble_output(run(in_maps))


# revision 26
# speedup vs baseline: 1.1692x; 1.1692x over previous
"""Self-contained TRN2 Bass kernel for nn_EnhancedMultiheadAttention.

kernel(**inputs) takes the FULL unsharded inputs (x, Wq, bq, Wk, bk, Wv, bv,
Wo, bo as float32 numpy arrays), distributes the computation across 8
NeuronCores (tensor-parallel over heads: core c owns heads 2c, 2c+1), and
returns the full [2, 2048, 1024] float32 output.

v2 design notes:
- all matmul operands in bf16 (fast weight load, half DMA/SBUF traffic);
  accumulation stays fp32 in PSUM.
- QK^T packs both local heads into one PE pass via 64-row tile_position
  row-tiles running concurrently (heads stacked on partitions 0:64/64:128).
- V projection is computed directly in [token, dim] layout (lhsT=x chunk)
  so no PE transposes are needed; a ones-column rides along for the
  softmax denominator.
- output tokens are sharded interleaved per 1024-token block (core c owns
  tokens blk*1024 + c*128 ..+128) so one small bf16 AllToAll fires per
  block and overlaps the remaining attention; output projection runs in
  the tail, overlapping the last A2A.
- softmax denominator reciprocal uses reciprocal_approx_fast + a DRAM
  bounce row for the partition broadcast.
"""

import sys

for _p in ("/opt/trn_rl_repo", "/root/.axon_site/_ro/trn_rl_repo"):
    if _p not in sys.path:
        sys.path.append(_p)


import numpy as np

import concourse.bass as bass
import concourse.mybir as mybir
import concourse.tile as tile
import bass_rust

F32 = mybir.dt.float32
B16 = mybir.dt.bfloat16

B, L, D = 2, 2048, 1024
H, DH = 16, 64
NCORES = 8
T = B * L                  # 4096 flattened tokens
NKC = D // 128             # 8 contraction chunks of 128
CHUNK = 1024               # projection token-chunk width
IB = 1024                  # query block width
NI = L // IB               # 2 query blocks per batch
NJ = L // 128              # 16 key chunks of 128 per batch
HPC = H // NCORES          # 2 heads per core
NBLK = T // IB             # 4 token blocks (A2A granularity)
TPC_BLK = IB // NCORES     # 128 tokens per core per block
TC = NBLK * TPC_BLK        # 512 output tokens per core


def split_excess_waits(nc, max_waits=1):
    """walrus's setupSyncWait rejects instructions with more than one wait
    condition on this compiler version; hoist extras onto preceding NoOps."""
    n_split = 0
    for f in nc.m.functions:
        for b in f.blocks:
            new_list = None
            for inst in list(b.instructions):
                si = inst.sync_info
                if si is None or len(si.on_wait) <= max_waits:
                    continue
                waits = list(si.on_wait)
                keep = waits[-max_waits:]
                excess = waits[:-max_waits]
                nops = []
                for j, w in enumerate(excess):
                    nop = mybir.InstNoOp(
                        name=f"I-wsplit-{inst.name}-{j}", ins=[], outs=[],
                        engine=inst.engine,
                    )
                    nop.sync_info = bass_rust.SyncInfo(on_wait=[w], on_update=[])
                    nops.append(nop)
                inst.sync_info = bass_rust.SyncInfo(
                    on_wait=keep, on_update=list(si.on_update)
                )
                if new_list is None:
                    new_list = list(b.instructions)
                pos = new_list.index(inst)
                new_list[pos:pos] = nops
                n_split += 1
            if new_list is not None:
                b.instructions = new_list
    return n_split


def build_nc(proj_mm=None, attn_mm=None, out_mm=None):
    nc = bass.Bass("TRN2", target_bir_lowering=False, debug=False,
                   num_devices=NCORES)

    xT = nc.dram_tensor("xT", [D, T], B16, kind="ExternalInput").ap()
    wq = nc.dram_tensor("wq", [D, 128], B16, kind="ExternalInput").ap()
    wk = nc.dram_tensor("wk", [D, 128], B16, kind="ExternalInput").ap()
    wv = nc.dram_tensor("wv", [D, 128], B16, kind="ExternalInput").ap()
    bq = nc.dram_tensor("bq", [128, 1], F32, kind="ExternalInput").ap()
    bk = nc.dram_tensor("bk", [128, 1], F32, kind="ExternalInput").ap()
    bv = nc.dram_tensor("bv", [128, 1], F32, kind="ExternalInput").ap()
    wo = nc.dram_tensor("wo", [D, D], B16, kind="ExternalInput").ap()
    bo = nc.dram_tensor("bo", [1, D], F32, kind="ExternalInput").ap()
    cosT = nc.dram_tensor("cosT", [128, L], B16, kind="ExternalInput").ap()
    sinT = nc.dram_tensor("sinT", [128, L], B16, kind="ExternalInput").ap()
    out = nc.dram_tensor("out", [TC, D], F32, kind="ExternalOutput").ap()

    with tile.TileContext(nc) as tc:
        _build_body(nc, tc, xT, wq, wk, wv, bq, bk, bv, wo, bo, cosT, sinT,
                    out)

    split_excess_waits(nc)
    return nc


def _build_body(nc, tc, xT, wq, wk, wv, bq, bk, bv, wo, bo, cosT, sinT, out):
    from contextlib import ExitStack

    ctx = ExitStack()
    with ctx:
        # ---------------- persistent tensors ----------------
        persist = ctx.enter_context(tc.tile_pool(name="persist", bufs=1))
        wpool = ctx.enter_context(tc.tile_pool(name="wqkv", bufs=1))

        w_t = {}
        b_t = {}
        for name, wap, bap in (("q", wq, bq), ("k", wk, bk), ("v", wv, bv)):
            w_t[name] = wpool.tile([128, NKC, 128], B16, tag=f"w{name}",
                                   name=f"w{name}")
            nc.sync.dma_start(w_t[name][:],
                              wap.rearrange("(kc p) m -> p kc m", p=128))
            b_t[name] = wpool.tile([128, 1], F32, tag=f"b{name}",
                                   name=f"b{name}")
            nc.sync.dma_start(b_t[name][:], bap)

        cos_l = persist.tile([128, L], B16, tag="cos", name="cos")
        sin_l = persist.tile([128, L], B16, tag="sin", name="sin")
        nc.sync.dma_start(cos_l[:], cosT[:])
        nc.sync.dma_start(sin_l[:], sinT[:])

        ones_col = persist.tile([128, 1], F32, tag="ones", name="ones")
        nc.gpsimd.memset(ones_col[:], 1.0)

        ident = persist.tile([128, 128], B16, tag="ident", name="ident")
        from concourse.masks import make_identity
        make_identity(nc, ident[:])

        # per-batch Q/K (heads stacked on partitions) and V ([token, dim])
        qt_sb = [persist.tile([128, L], B16, tag=f"qt{b}", name=f"qt{b}")
                 for b in range(B)]
        kt_sb = [persist.tile([128, L], B16, tag=f"kt{b}", name=f"kt{b}")
                 for b in range(B)]
        v_sb = [[persist.tile([128, NJ, 66], B16, tag=f"v{b}{h}",
                              name=f"v{b}{h}")
                 for h in range(HPC)] for b in range(B)]
        for b in range(B):
            for h in range(HPC):
                nc.vector.tensor_copy(
                    v_sb[b][h][:, :, 64:65],
                    ones_col[:, :].to_broadcast((128, NJ, 1)))

        # context after A2A: [128 dims-of-kc, kc, blk, tok]
        opool = ctx.enter_context(tc.tile_pool(name="oproj", bufs=1))
        wo_sb = opool.tile([128, NKC, D], B16, tag="wo", name="wo")
        bo_sb = opool.tile([128, D], F32, tag="bo", name="bo")
        ctx_sb = opool.tile([128, NKC, NBLK, TPC_BLK], B16, tag="ctxsb",
                            name="ctxsb")

        # DRAM buffers for the per-block collectives (bf16)
        dram = ctx.enter_context(tc.tile_pool(name="dram", bufs=1,
                                              space="DRAM"))
        a2a_in = [dram.tile([NCORES * DH, HPC * TPC_BLK], B16,
                            name=f"a2a_in{blk}") for blk in range(NBLK)]
        a2a_out = [dram.tile([NCORES * DH, HPC * TPC_BLK], B16,
                             name=f"a2a_out{blk}") for blk in range(NBLK)]

        xT3 = xT.rearrange("(kc p) t -> p kc t", p=128)

        # prefetch ALL x chunks up front (sync queue) so the inter-batch
        # transition never waits on HBM
        xpool = ctx.enter_context(tc.tile_pool(name="x", bufs=4))
        xt_c = []
        for ci in range(T // CHUNK):
            xt = xpool.tile([128, NKC, CHUNK], B16, tag="xchunk",
                            name=f"xchunk{ci}")
            for kc in range(NKC):
                nc.sync.dma_start(xt[:, kc, :], xT3[:, kc, bass.ts(ci, CHUNK)])
            xt_c.append(xt)
        # big output-projection weights load overlaps everything
        nc.sync.dma_start(wo_sb[:],
                          wo.rearrange("(kc p) n -> p kc n", p=128))
        nc.sync.dma_start(bo_sb[:], bo.to_broadcast((128, D)))

        # ---------------- phase A: projections + RoPE (all chunks) -------
        actx = ExitStack()
        tmp = actx.enter_context(tc.tile_pool(name="ptmp", bufs=3))
        ppsum = actx.enter_context(
            tc.tile_pool(name="ppsum", bufs=2, space="PSUM"))
        vtpsum = actx.enter_context(
            tc.tile_pool(name="vtpsum", bufs=2, space="PSUM"))

        for ci in range(T // CHUNK):
            b, half = divmod(ci, L // CHUNK)
            if True:
                lsl = bass.ts(half, CHUNK)         # slice into qt/kt/cos/sin
                xt = xt_c[ci]
                cos_t = cos_l[:, lsl]
                sin_t = sin_l[:, lsl]

                # Q/K/V projections: [dims, tokens]
                for name in ("q", "k", "v"):
                    ps = ppsum.tile([128, CHUNK], F32, tag="proj", name="proj")
                    for kc in range(NKC):
                        for nh in range(CHUNK // 512):
                            nc.tensor.matmul(
                                ps[:, bass.ts(nh, 512)],
                                w_t[name][:, kc, :],
                                xt[:, kc, bass.ts(nh, 512)],
                                start=(kc == 0), stop=(kc == NKC - 1),
                            )
                    if name == "v":
                        raw = tmp.tile([128, CHUNK], B16, tag="raw",
                                       name="raw")
                        nc.vector.tensor_scalar_add(raw[:], ps[:], b_t["v"][:])
                        # transpose [64, 128] blocks into [token, dim] via PE
                        for h in range(HPC):
                            o = h * DH
                            for sub in range(CHUNK // 128):
                                kb = half * (CHUNK // 128) + sub
                                tp = vtpsum.tile([128, DH], B16, tag="vt",
                                                 name="vt")
                                nc.tensor.transpose(
                                    tp[:],
                                    raw[bass.ds(o, DH), bass.ts(sub, 128)],
                                    ident[bass.ds(o, DH), bass.ds(o, DH)],
                                )
                                nc.vector.tensor_copy(
                                    v_sb[b][h][:, kb, 0:DH], tp[:])
                        continue
                    # Q/K: bias then RoPE
                    raw = tmp.tile([128, CHUNK], B16, tag="raw", name="raw")
                    nc.vector.tensor_scalar_add(raw[:], ps[:], b_t[name][:])
                    shifted = tmp.tile([128, CHUNK], B16, tag="shift",
                                       name="shift")
                    for h in range(HPC):
                        o = h * DH
                        nc.scalar.dma_start(shifted[o:o + 32, :],
                                            raw[o + 32:o + 64, :])
                        nc.scalar.dma_start(shifted[o + 32:o + 64, :],
                                            raw[o:o + 32, :])
                    t1 = tmp.tile([128, CHUNK], B16, tag="t1", name="t1")
                    nc.vector.tensor_mul(t1[:], raw[:], cos_t)
                    nc.vector.tensor_mul(shifted[:], shifted[:], sin_t)
                    dst = qt_sb[b] if name == "q" else kt_sb[b]
                    nc.vector.tensor_add(dst[:, lsl], t1[:], shifted[:])

        actx.close()

        # ---------------- phase B: attention (all blocks) ----------------
        bctx = ExitStack()
        ppool = bctx.enter_context(tc.tile_pool(name="pT", bufs=4))
        avspool = bctx.enter_context(tc.tile_pool(name="avs", bufs=2))
        rrpool = bctx.enter_context(tc.tile_pool(name="rr", bufs=2))
        rbpool = bctx.enter_context(tc.tile_pool(name="rb", bufs=2))
        cxpool = bctx.enter_context(tc.tile_pool(name="cx", bufs=2))
        stpsum = bctx.enter_context(
            tc.tile_pool(name="stpsum", bufs=2, space="PSUM"))
        avpsum = bctx.enter_context(
            tc.tile_pool(name="avpsum", bufs=2, space="PSUM"))
        ndram = bctx.enter_context(
            tc.tile_pool(name="ndram", bufs=2, space="DRAM"))

        for blk in range(NBLK):
            b, ib = divmod(blk, NI)
            if True:
                av = [avpsum.tile([128, IB], F32, tag="av", name="av")
                      for _ in range(HPC)]
                pt_prev = None
                for jc in range(NJ):
                    st = [stpsum.tile([128, IB], F32, tag="st", name="st")
                          for _ in range(HPC)]
                    ksl = bass.ds(jc * 128, 128)
                    # QK^T for both heads concurrently (64-row PE tiles)
                    for nh in range(IB // 512):
                        qsl = bass.ds(ib * IB + nh * 512, 512)
                        for h in range(HPC):
                            o = h * DH
                            nc.tensor.matmul(
                                st[h][:, bass.ts(nh, 512)],
                                kt_sb[b][o:o + DH, ksl],
                                qt_sb[b][o:o + DH, qsl],
                                start=True, stop=True,
                            )
                    pt = [ppool.tile([128, IB], B16, tag="pt", name="pt")
                          for _ in range(HPC)]
                    for h in range(HPC):
                        nc.scalar.activation(pt[h][:], st[h][:],
                                             mybir.ActivationFunctionType.Exp,
                                             scale=float(DH) ** -0.5)
                    if pt_prev is not None:
                        _emit_av(nc, av, v_sb[b], pt_prev, jc - 1)
                    pt_prev = pt
                _emit_av(nc, av, v_sb[b], pt_prev, NJ - 1)

                # normalize: den is row DH of av. Bounce it through DRAM
                # reshaped to [128, IB//128] so reciprocal runs on all 128
                # DVE lanes (the [1, IB] form is ~8 cyc/elem on one lane),
                # then bounce back and broadcast-read over DH partitions.
                for h in range(HPC):
                    avs = avspool.tile([DH + 1, IB], F32, tag="avs",
                                       name="avs")
                    nc.vector.tensor_copy(avs[:], av[h][0:DH + 1, :])
                    rr = rrpool.tile([DH + 1, IB], F32, tag="rr", name="rr")
                    nc.vector.reciprocal(rr[DH:DH + 1, :], avs[DH:DH + 1, :])
                    rrow = ndram.tile([1, IB], F32, tag="rrow", name="rrow")
                    nc.sync.dma_start(rrow[:], rr[DH:DH + 1, :])
                    rb = rbpool.tile([DH, IB], F32, tag="rb", name="rb")
                    nc.sync.dma_start(rb[:], rrow[0:1, :].to_broadcast((DH, IB)))
                    cx = cxpool.tile([DH, IB], B16, tag="cx", name="cx")
                    nc.vector.tensor_mul(cx[:], avs[0:DH, :], rb[:])
                    # stage into the A2A buffer: shard s gets this head's
                    # tokens s*128..+128 (cols h*128..+128 of the shard)
                    for s in range(NCORES):
                        nc.sync.dma_start(
                            a2a_in[blk][bass.ds(s * DH, DH),
                                        bass.ds(h * TPC_BLK, TPC_BLK)],
                            cx[:, bass.ts(s, TPC_BLK)])

                nc.gpsimd.collective_compute(
                    "AllToAll",
                    mybir.AluOpType.bypass,
                    replica_groups=[list(range(NCORES))],
                    ins=[a2a_in[blk][:]],
                    outs=[a2a_out[blk][:]],
                )
                for h in range(HPC):
                    src = a2a_out[blk][:, bass.ds(h * TPC_BLK, TPC_BLK)] \
                        .rearrange("(g p) t -> p g t", p=DH)
                    nc.gpsimd.dma_start(
                        ctx_sb[bass.ds(h * DH, DH), :, blk, :], src)

        bctx.close()

        # ---------------- phase C: output projection (tail) ----------------
        ostage = ctx.enter_context(tc.tile_pool(name="ostage", bufs=2))
        opsum = ctx.enter_context(tc.tile_pool(name="opsum", bufs=4,
                                               space="PSUM"))

        for blk in range(NBLK):
            pss = [opsum.tile([128, 512], F32, tag="ops", name=f"ops{blk}{nh}")
                   for nh in range(2)]
            for kc in range(NKC):
                for nh in range(2):
                    nc.tensor.matmul(
                        pss[nh][:],
                        ctx_sb[:, kc, blk, :],
                        wo_sb[:, kc, bass.ts(nh, 512)],
                        start=(kc == 0), stop=(kc == NKC - 1),
                    )
            for nh in range(2):
                ot = ostage.tile([128, 512], F32, tag="ot", name="ot")
                nc.vector.tensor_add(ot[:], pss[nh][:],
                                     bo_sb[:, bass.ts(nh, 512)])
                nc.sync.dma_start(
                    out[bass.ts(blk, TPC_BLK), bass.ts(nh, 512)], ot[:])


def _emit_av(nc, av, v_b, pt, jc):
    """AV accumulation for key chunk jc, both heads."""
    for h in range(HPC):
        for nh in range(IB // 512):
            nc.tensor.matmul(
                av[h][0:DH + 1, bass.ts(nh, 512)],
                v_b[h][:, jc, 0:DH + 1],
                pt[h][:, bass.ts(nh, 512)],
                start=(jc == 0), stop=(jc == NJ - 1),
            )


# ---------------- host-side sharding / unsharding ----------------

def _bf16(a):
    import ml_dtypes
    return np.ascontiguousarray(a).astype(ml_dtypes.bfloat16)


def rope_cos_sin_np(seq_len, d_head):
    inv_freq = 1.0 / (10000.0 ** (np.arange(0, d_head, 2, dtype=np.float32) / d_head))
    t = np.arange(seq_len, dtype=np.float32)
    freqs = np.einsum("i,j->ij", t, inv_freq).astype(np.float32)
    emb = np.concatenate((freqs, freqs), axis=-1)
    return np.cos(emb).astype(np.float32), np.sin(emb).astype(np.float32)


def make_in_maps(x, Wq, bq, Wk, bk, Wv, bv, Wo, bo):
    xT = _bf16(x.reshape(T, D).T)

    cos, sin = rope_cos_sin_np(L, DH)          # [L, 64]
    cosT = cos.T                               # [64, L]
    sinT = sin.T
    sgn = np.where(np.arange(DH) < DH // 2, -1.0, 1.0).astype(np.float32)
    sinT_signed = sinT * sgn[:, None]
    cosT_full = _bf16(np.tile(cosT, (HPC, 1)))      # [128, 2048]
    sinT_full = _bf16(np.tile(sinT_signed, (HPC, 1)))

    wo_full = _bf16(Wo)
    bo_row = np.ascontiguousarray(bo.reshape(1, D))

    in_maps = []
    for c in range(NCORES):
        sl = slice(c * 128, (c + 1) * 128)
        in_maps.append({
            "xT": xT,
            "wq": _bf16(Wq[:, sl]),
            "wk": _bf16(Wk[:, sl]),
            "wv": _bf16(Wv[:, sl]),
            "bq": np.ascontiguousarray(bq[sl].reshape(128, 1)),
            "bk": np.ascontiguousarray(bk[sl].reshape(128, 1)),
            "bv": np.ascontiguousarray(bv[sl].reshape(128, 1)),
            "wo": wo_full,
            "bo": bo_row,
            "cosT": cosT_full,
            "sinT": sinT_full,
        })
    return in_maps


def assemble_output(results):
    outs = np.stack([results[c]["out"] for c in range(NCORES)])  # [8, 512, D]
    full = outs.reshape(NCORES, NBLK, TPC_BLK, D).transpose(1, 0, 2, 3)
    return np.ascontiguousarray(full.reshape(B, L, D)).astype(np.float32)


_CACHE = {}


def _get_runner():
    """Build the Bass program and a cached jitted SPMD executor once.

    Mirrors bass2jax.run_bass_via_pjrt's multi-core path, but keeps the
    jitted shard_map callable alive so repeat kernel() calls skip retracing.
    """
    if "runner" in _CACHE:
        return _CACHE["runner"]

    import jax
    import numpy as _np
    from jax.sharding import Mesh, PartitionSpec
    from jax.experimental.shard_map import shard_map
    from concourse import bass2jax, mybir as _mybir

    nc = build_nc()
    bass2jax.install_neuronx_cc_hook()

    partition_name = (nc.partition_id_tensor.name
                      if nc.partition_id_tensor else None)
    in_names, out_names, out_avals, zero_shapes = [], [], [], []
    for alloc in nc.m.functions[0].allocations:
        if not isinstance(alloc, _mybir.MemoryLocationSet):
            continue
        name = alloc.memorylocations[0].name
        if alloc.kind == "ExternalInput":
            if name != partition_name:
                in_names.append(name)
        elif alloc.kind == "ExternalOutput":
            shape = tuple(alloc.tensor_shape)
            dtype = _mybir.dt.np(alloc.dtype)
            out_names.append(name)
            out_avals.append(jax.core.ShapedArray(shape, dtype))
            zero_shapes.append((shape, dtype))
    n_params = len(in_names)
    n_outs = len(out_avals)
    all_in_names = list(in_names) + list(out_names)
    if partition_name is not None:
        all_in_names.append(partition_name)
    donate = tuple(range(n_params, n_params + n_outs))

    def _body(*args):
        operands = list(args)
        if partition_name is not None:
            operands.append(bass2jax.partition_id_tensor())
        outs = bass2jax._bass_exec_p.bind(
            *operands,
            out_avals=tuple(out_avals),
            in_names=tuple(all_in_names),
            out_names=tuple(out_names),
            lowering_input_output_aliases=(),
            sim_require_finite=True,
            sim_require_nnan=True,
            nc=nc,
        )
        return tuple(outs)

    devices = jax.devices()[:NCORES]
    mesh = Mesh(_np.asarray(devices), ("core",))
    in_specs = (PartitionSpec("core"),) * (n_params + n_outs)
    out_specs = (PartitionSpec("core"),) * n_outs
    sharded = jax.jit(
        shard_map(_body, mesh=mesh, in_specs=in_specs, out_specs=out_specs,
                  check_rep=False),
        donate_argnums=donate,
        keep_unused=True,
    )

    def run(in_maps):
        per_core = [[_np.asarray(m[name]) for name in in_names]
                    for m in in_maps]
        concat_in = [
            _np.concatenate([per_core[c][i] for c in range(NCORES)], axis=0)
            for i in range(n_params)
        ]
        concat_zeros = [
            _np.zeros((NCORES * s[0], *s[1:]), dt) for s, dt in zero_shapes
        ]
        out_arrs = sharded(*concat_in, *concat_zeros)
        return [
            {name: _np.asarray(out_arrs[i]).reshape(
                NCORES, *out_avals[i].shape)[c]
             for i, name in enumerate(out_names)}
            for c in range(NCORES)
        ]

    _CACHE["runner"] = run
    return run


def kernel(**inputs):
    run = _get_runner()
    in_maps = make_in_maps(**{k: np.asarray(v, dtype=np.float32)
                              for k, v in inputs.items()})
    return assemble_output(run(in_maps))


# revision 28
# speedup vs baseline: 1.3066x; 1.1175x over previous
"""Self-contained TRN2 Bass kernel for nn_EnhancedMultiheadAttention.

kernel(**inputs) takes the FULL unsharded inputs (x, Wq, bq, Wk, bk, Wv, bv,
Wo, bo as float32 numpy arrays), distributes the computation across 8
NeuronCores (tensor-parallel over heads: core c owns heads 2c, 2c+1), and
returns the full [2, 2048, 1024] float32 output.

v2 design notes:
- all matmul operands in bf16 (fast weight load, half DMA/SBUF traffic);
  accumulation stays fp32 in PSUM.
- QK^T packs both local heads into one PE pass via 64-row tile_position
  row-tiles running concurrently (heads stacked on partitions 0:64/64:128).
- V projection is computed directly in [token, dim] layout (lhsT=x chunk)
  so no PE transposes are needed; a ones-column rides along for the
  softmax denominator.
- output tokens are sharded interleaved per 1024-token block (core c owns
  tokens blk*1024 + c*128 ..+128) so one small bf16 AllToAll fires per
  block and overlaps the remaining attention; output projection runs in
  the tail, overlapping the last A2A.
- softmax denominator reciprocal uses reciprocal_approx_fast + a DRAM
  bounce row for the partition broadcast.
"""

import sys

for _p in ("/opt/trn_rl_repo", "/root/.axon_site/_ro/trn_rl_repo"):
    if _p not in sys.path:
        sys.path.append(_p)


import numpy as np

import concourse.bass as bass
import concourse.mybir as mybir
import concourse.tile as tile
import bass_rust

F32 = mybir.dt.float32
B16 = mybir.dt.bfloat16

B, L, D = 2, 2048, 1024
H, DH = 16, 64
NCORES = 8
T = B * L                  # 4096 flattened tokens
NKC = D // 128             # 8 contraction chunks of 128
CHUNK = 1024               # projection token-chunk width
IB = 1024                  # query block width
NI = L // IB               # 2 query blocks per batch
NJ = L // 128              # 16 key chunks of 128 per batch
HPC = H // NCORES          # 2 heads per core
NBLK = T // IB             # 4 token blocks (A2A granularity)
TPC_BLK = IB // NCORES     # 128 tokens per core per block
TC = NBLK * TPC_BLK        # 512 output tokens per core


def split_excess_waits(nc, max_waits=1):
    """walrus's setupSyncWait rejects instructions with more than one wait
    condition on this compiler version; hoist extras onto preceding NoOps."""
    n_split = 0
    for f in nc.m.functions:
        for b in f.blocks:
            new_list = None
            for inst in list(b.instructions):
                si = inst.sync_info
                if si is None or len(si.on_wait) <= max_waits:
                    continue
                waits = list(si.on_wait)
                keep = waits[-max_waits:]
                excess = waits[:-max_waits]
                nops = []
                for j, w in enumerate(excess):
                    nop = mybir.InstNoOp(
                        name=f"I-wsplit-{inst.name}-{j}", ins=[], outs=[],
                        engine=inst.engine,
                    )
                    nop.sync_info = bass_rust.SyncInfo(on_wait=[w], on_update=[])
                    nops.append(nop)
                inst.sync_info = bass_rust.SyncInfo(
                    on_wait=keep, on_update=list(si.on_update)
                )
                if new_list is None:
                    new_list = list(b.instructions)
                pos = new_list.index(inst)
                new_list[pos:pos] = nops
                n_split += 1
            if new_list is not None:
                b.instructions = new_list
    return n_split


def build_nc(proj_mm=None, attn_mm=None, out_mm=None):
    nc = bass.Bass("TRN2", target_bir_lowering=False, debug=False,
                   num_devices=NCORES)

    xT = nc.dram_tensor("xT", [D, T], B16, kind="ExternalInput").ap()
    wq = nc.dram_tensor("wq", [D, 128], B16, kind="ExternalInput").ap()
    wk = nc.dram_tensor("wk", [D, 128], B16, kind="ExternalInput").ap()
    wv = nc.dram_tensor("wv", [D, 128], B16, kind="ExternalInput").ap()
    bq = nc.dram_tensor("bq", [128, 1], F32, kind="ExternalInput").ap()
    bk = nc.dram_tensor("bk", [128, 1], F32, kind="ExternalInput").ap()
    bv = nc.dram_tensor("bv", [128, 1], F32, kind="ExternalInput").ap()
    wo = nc.dram_tensor("wo", [D, D], B16, kind="ExternalInput").ap()
    bo = nc.dram_tensor("bo", [1, D], F32, kind="ExternalInput").ap()
    cosT = nc.dram_tensor("cosT", [128, L], B16, kind="ExternalInput").ap()
    sinT = nc.dram_tensor("sinT", [128, L], B16, kind="ExternalInput").ap()
    out = nc.dram_tensor("out", [TC, D], F32, kind="ExternalOutput").ap()

    with tile.TileContext(nc) as tc:
        _build_body(nc, tc, xT, wq, wk, wv, bq, bk, bv, wo, bo, cosT, sinT,
                    out)

    split_excess_waits(nc)
    return nc


def _build_body(nc, tc, xT, wq, wk, wv, bq, bk, bv, wo, bo, cosT, sinT, out):
    from contextlib import ExitStack

    ctx = ExitStack()
    with ctx:
        # ---------------- persistent tensors ----------------
        persist = ctx.enter_context(tc.tile_pool(name="persist", bufs=1))
        wpool = ctx.enter_context(tc.tile_pool(name="wqkv", bufs=1))

        w_t = {}
        b_t = {}
        for name, wap, bap in (("q", wq, bq), ("k", wk, bk), ("v", wv, bv)):
            w_t[name] = wpool.tile([128, NKC, 128], B16, tag=f"w{name}",
                                   name=f"w{name}")
            nc.sync.dma_start(w_t[name][:],
                              wap.rearrange("(kc p) m -> p kc m", p=128))
            b_t[name] = wpool.tile([128, 1], F32, tag=f"b{name}",
                                   name=f"b{name}")
            nc.sync.dma_start(b_t[name][:], bap)

        cos_l = persist.tile([128, L], B16, tag="cos", name="cos")
        sin_l = persist.tile([128, L], B16, tag="sin", name="sin")
        nc.sync.dma_start(cos_l[:], cosT[:])
        nc.sync.dma_start(sin_l[:], sinT[:])

        ones_col = persist.tile([128, 1], F32, tag="ones", name="ones")
        nc.gpsimd.memset(ones_col[:], 1.0)

        ident = persist.tile([128, 128], B16, tag="ident", name="ident")
        from concourse.masks import make_identity
        make_identity(nc, ident[:])

        # per-batch Q/K (heads stacked on partitions) and V ([token, dim])
        qt_sb = [persist.tile([128, L], B16, tag=f"qt{b}", name=f"qt{b}")
                 for b in range(B)]
        kt_sb = [persist.tile([128, L], B16, tag=f"kt{b}", name=f"kt{b}")
                 for b in range(B)]
        v_sb = [[persist.tile([128, NJ, 66], B16, tag=f"v{b}{h}",
                              name=f"v{b}{h}")
                 for h in range(HPC)] for b in range(B)]
        for b in range(B):
            for h in range(HPC):
                nc.vector.tensor_copy(
                    v_sb[b][h][:, :, 64:65],
                    ones_col[:, :].to_broadcast((128, NJ, 1)))

        # context after A2A: [128 dims-of-kc, kc, blk, tok]
        opool = ctx.enter_context(tc.tile_pool(name="oproj", bufs=1))
        wo_sb = opool.tile([128, NKC, D], B16, tag="wo", name="wo")
        bo_sb = opool.tile([128, D], F32, tag="bo", name="bo")
        ctx_sb = opool.tile([128, NKC, NBLK, TPC_BLK], B16, tag="ctxsb",
                            name="ctxsb")

        # DRAM buffers for the two paired-block collectives (bf16)
        dram = ctx.enter_context(tc.tile_pool(name="dram", bufs=1,
                                              space="DRAM"))
        PAIRW = 2 * HPC * TPC_BLK
        a2a_in = [dram.tile([NCORES * DH, PAIRW], B16,
                            name=f"a2a_in{g}") for g in range(2)]
        a2a_out = [dram.tile([NCORES * DH, PAIRW], B16,
                             name=f"a2a_out{g}") for g in range(2)]

        xT3 = xT.rearrange("(kc p) t -> p kc t", p=128)

        # prefetch ALL x chunks up front (sync queue) so the inter-batch
        # transition never waits on HBM
        xpool = ctx.enter_context(tc.tile_pool(name="x", bufs=4))
        xt_c = []
        for ci in range(T // CHUNK):
            xt = xpool.tile([128, NKC, CHUNK], B16, tag="xchunk",
                            name=f"xchunk{ci}")
            for kc in range(NKC):
                nc.sync.dma_start(xt[:, kc, :], xT3[:, kc, bass.ts(ci, CHUNK)])
            xt_c.append(xt)
        # big output-projection weights load overlaps everything
        nc.sync.dma_start(wo_sb[:],
                          wo.rearrange("(kc p) n -> p kc n", p=128))
        nc.sync.dma_start(bo_sb[:], bo.to_broadcast((128, D)))

        # ---------------- phase A: projections + RoPE (all chunks) -------
        actx = ExitStack()
        tmp = actx.enter_context(tc.tile_pool(name="ptmp", bufs=3))
        ppsum = actx.enter_context(
            tc.tile_pool(name="ppsum", bufs=2, space="PSUM"))
        vtpsum = actx.enter_context(
            tc.tile_pool(name="vtpsum", bufs=2, space="PSUM"))

        for ci in range(T // CHUNK):
            b, half = divmod(ci, L // CHUNK)
            if True:
                lsl = bass.ts(half, CHUNK)         # slice into qt/kt/cos/sin
                xt = xt_c[ci]
                cos_t = cos_l[:, lsl]
                sin_t = sin_l[:, lsl]

                # Q/K/V projections: [dims, tokens]
                for name in ("q", "k", "v"):
                    ps = ppsum.tile([128, CHUNK], F32, tag="proj", name="proj")
                    for kc in range(NKC):
                        for nh in range(CHUNK // 512):
                            nc.tensor.matmul(
                                ps[:, bass.ts(nh, 512)],
                                w_t[name][:, kc, :],
                                xt[:, kc, bass.ts(nh, 512)],
                                start=(kc == 0), stop=(kc == NKC - 1),
                            )
                    if name == "v":
                        raw = tmp.tile([128, CHUNK], B16, tag="raw",
                                       name="raw")
                        nc.vector.tensor_scalar_add(raw[:], ps[:], b_t["v"][:])
                        # transpose [64, 128] blocks into [token, dim] via PE
                        for h in range(HPC):
                            o = h * DH
                            for sub in range(CHUNK // 128):
                                kb = half * (CHUNK // 128) + sub
                                tp = vtpsum.tile([128, DH], B16, tag="vt",
                                                 name="vt")
                                nc.tensor.transpose(
                                    tp[:],
                                    raw[bass.ds(o, DH), bass.ts(sub, 128)],
                                    ident[bass.ds(o, DH), bass.ds(o, DH)],
                                )
                                nc.vector.tensor_copy(
                                    v_sb[b][h][:, kb, 0:DH], tp[:])
                        continue
                    # Q/K: bias then RoPE
                    raw = tmp.tile([128, CHUNK], B16, tag="raw", name="raw")
                    nc.vector.tensor_scalar_add(raw[:], ps[:], b_t[name][:])
                    shifted = tmp.tile([128, CHUNK], B16, tag="shift",
                                       name="shift")
                    for h in range(HPC):
                        o = h * DH
                        nc.scalar.dma_start(shifted[o:o + 32, :],
                                            raw[o + 32:o + 64, :])
                        nc.scalar.dma_start(shifted[o + 32:o + 64, :],
                                            raw[o:o + 32, :])
                    t1 = tmp.tile([128, CHUNK], B16, tag="t1", name="t1")
                    nc.vector.tensor_mul(t1[:], raw[:], cos_t)
                    nc.vector.tensor_mul(shifted[:], shifted[:], sin_t)
                    dst = qt_sb[b] if name == "q" else kt_sb[b]
                    nc.vector.tensor_add(dst[:, lsl], t1[:], shifted[:])

        actx.close()

        # ---------------- phase B: attention (all blocks) ----------------
        bctx = ExitStack()
        ppool = bctx.enter_context(tc.tile_pool(name="pT", bufs=4))
        avspool = bctx.enter_context(tc.tile_pool(name="avs", bufs=2))
        rrpool = bctx.enter_context(tc.tile_pool(name="rr", bufs=2))
        rbpool = bctx.enter_context(tc.tile_pool(name="rb", bufs=2))
        cxpool = bctx.enter_context(tc.tile_pool(name="cx", bufs=2))
        stpsum = bctx.enter_context(
            tc.tile_pool(name="stpsum", bufs=2, space="PSUM"))
        avpsum = bctx.enter_context(
            tc.tile_pool(name="avpsum", bufs=2, space="PSUM"))
        ndram = bctx.enter_context(
            tc.tile_pool(name="ndram", bufs=2, space="DRAM"))

        for blk in range(NBLK):
            b, ib = divmod(blk, NI)
            if True:
                av = [avpsum.tile([128, IB], F32, tag="av", name="av")
                      for _ in range(HPC)]
                pt_prev = None
                for jc in range(NJ):
                    st = [stpsum.tile([128, IB], F32, tag="st", name="st")
                          for _ in range(HPC)]
                    ksl = bass.ds(jc * 128, 128)
                    # QK^T for both heads concurrently (64-row PE tiles)
                    for nh in range(IB // 512):
                        qsl = bass.ds(ib * IB + nh * 512, 512)
                        for h in range(HPC):
                            o = h * DH
                            nc.tensor.matmul(
                                st[h][:, bass.ts(nh, 512)],
                                kt_sb[b][o:o + DH, ksl],
                                qt_sb[b][o:o + DH, qsl],
                                start=True, stop=True,
                            )
                    pt = [ppool.tile([128, IB], B16, tag="pt", name="pt")
                          for _ in range(HPC)]
                    for h in range(HPC):
                        nc.scalar.activation(pt[h][:], st[h][:],
                                             mybir.ActivationFunctionType.Exp,
                                             scale=float(DH) ** -0.5)
                    if pt_prev is not None:
                        _emit_av(nc, av, v_sb[b], pt_prev, jc - 1)
                    pt_prev = pt
                _emit_av(nc, av, v_sb[b], pt_prev, NJ - 1)

                # normalize: den is row DH of av. Bounce it through DRAM
                # reshaped to [128, IB//128] so reciprocal runs on all 128
                # DVE lanes (the [1, IB] form is ~8 cyc/elem on one lane),
                # then bounce back and broadcast-read over DH partitions.
                for h in range(HPC):
                    avs = avspool.tile([DH + 1, IB], F32, tag="avs",
                                       name="avs")
                    nc.vector.tensor_copy(avs[:], av[h][0:DH + 1, :])
                    rr = rrpool.tile([DH + 1, IB], F32, tag="rr", name="rr")
                    nc.vector.reciprocal(rr[DH:DH + 1, :], avs[DH:DH + 1, :])
                    rrow = ndram.tile([1, IB], F32, tag="rrow", name="rrow")
                    nc.sync.dma_start(rrow[:], rr[DH:DH + 1, :])
                    rb = rbpool.tile([DH, IB], F32, tag="rb", name="rb")
                    nc.sync.dma_start(rb[:], rrow[0:1, :].to_broadcast((DH, IB)))
                    cx = cxpool.tile([DH, IB], B16, tag="cx", name="cx")
                    nc.vector.tensor_mul(cx[:], avs[0:DH, :], rb[:])
                    # stage into the pair A2A buffer: shard s gets this
                    # head's tokens s*128..+128 of this block
                    g = blk // 2
                    coff = (blk % 2) * HPC * TPC_BLK + h * TPC_BLK
                    for s in range(NCORES):
                        nc.sync.dma_start(
                            a2a_in[g][bass.ds(s * DH, DH),
                                      bass.ds(coff, TPC_BLK)],
                            cx[:, bass.ts(s, TPC_BLK)])

                if blk % 2 == 1:
                    g = blk // 2
                    nc.gpsimd.collective_compute(
                        "AllToAll",
                        mybir.AluOpType.bypass,
                        replica_groups=[list(range(NCORES))],
                        ins=[a2a_in[g][:]],
                        outs=[a2a_out[g][:]],
                    )
                    for pb in range(2):
                        rblk = 2 * g + pb
                        for h in range(HPC):
                            coff = pb * HPC * TPC_BLK + h * TPC_BLK
                            rsrc = a2a_out[g][:, bass.ds(coff, TPC_BLK)] \
                                .rearrange("(gg p) t -> p gg t", p=DH)
                            nc.gpsimd.dma_start(
                                ctx_sb[bass.ds(h * DH, DH), :, rblk, :], rsrc)

        bctx.close()

        # ---------------- phase C: output projection (tail) ----------------
        ostage = ctx.enter_context(tc.tile_pool(name="ostage", bufs=2))
        opsum = ctx.enter_context(tc.tile_pool(name="opsum", bufs=4,
                                               space="PSUM"))

        for blk in range(NBLK):
            pss = [opsum.tile([128, 512], F32, tag="ops", name=f"ops{blk}{nh}")
                   for nh in range(2)]
            for kc in range(NKC):
                for nh in range(2):
                    nc.tensor.matmul(
                        pss[nh][:],
                        ctx_sb[:, kc, blk, :],
                        wo_sb[:, kc, bass.ts(nh, 512)],
                        start=(kc == 0), stop=(kc == NKC - 1),
                    )
            for nh in range(2):
                ot = ostage.tile([128, 512], F32, tag="ot", name="ot")
                nc.vector.tensor_add(ot[:], pss[nh][:],
                                     bo_sb[:, bass.ts(nh, 512)])
                nc.sync.dma_start(
                    out[bass.ts(blk, TPC_BLK), bass.ts(nh, 512)], ot[:])


def _emit_av(nc, av, v_b, pt, jc):
    """AV accumulation for key chunk jc, both heads."""
    for h in range(HPC):
        for nh in range(IB // 512):
            nc.tensor.matmul(
                av[h][0:DH + 1, bass.ts(nh, 512)],
                v_b[h][:, jc, 0:DH + 1],
                pt[h][:, bass.ts(nh, 512)],
                start=(jc == 0), stop=(jc == NJ - 1),
            )


# ---------------- host-side sharding / unsharding ----------------

def _bf16(a):
    import ml_dtypes
    return np.ascontiguousarray(a).astype(ml_dtypes.bfloat16)


def rope_cos_sin_np(seq_len, d_head):
    inv_freq = 1.0 / (10000.0 ** (np.arange(0, d_head, 2, dtype=np.float32) / d_head))
    t = np.arange(seq_len, dtype=np.float32)
    freqs = np.einsum("i,j->ij", t, inv_freq).astype(np.float32)
    emb = np.concatenate((freqs, freqs), axis=-1)
    return np.cos(emb).astype(np.float32), np.sin(emb).astype(np.float32)


def make_in_maps(x, Wq, bq, Wk, bk, Wv, bv, Wo, bo):
    xT = _bf16(x.reshape(T, D).T)

    cos, sin = rope_cos_sin_np(L, DH)          # [L, 64]
    cosT = cos.T                               # [64, L]
    sinT = sin.T
    sgn = np.where(np.arange(DH) < DH // 2, -1.0, 1.0).astype(np.float32)
    sinT_signed = sinT * sgn[:, None]
    cosT_full = _bf16(np.tile(cosT, (HPC, 1)))      # [128, 2048]
    sinT_full = _bf16(np.tile(sinT_signed, (HPC, 1)))

    wo_full = _bf16(Wo)
    bo_row = np.ascontiguousarray(bo.reshape(1, D))

    in_maps = []
    for c in range(NCORES):
        sl = slice(c * 128, (c + 1) * 128)
        in_maps.append({
            "xT": xT,
            "wq": _bf16(Wq[:, sl]),
            "wk": _bf16(Wk[:, sl]),
            "wv": _bf16(Wv[:, sl]),
            "bq": np.ascontiguousarray(bq[sl].reshape(128, 1)),
            "bk": np.ascontiguousarray(bk[sl].reshape(128, 1)),
            "bv": np.ascontiguousarray(bv[sl].reshape(128, 1)),
            "wo": wo_full,
            "bo": bo_row,
            "cosT": cosT_full,
            "sinT": sinT_full,
        })
    return in_maps


def assemble_output(results):
    outs = np.stack([results[c]["out"] for c in range(NCORES)])  # [8, 512, D]
    full = outs.reshape(NCORES, NBLK, TPC_BLK, D).transpose(1, 0, 2, 3)
    return np.ascontiguousarray(full.reshape(B, L, D)).astype(np.float32)


_CACHE = {}


def _get_runner():
    """Build the Bass program and a cached jitted SPMD executor once.

    Mirrors bass2jax.run_bass_via_pjrt's multi-core path, but keeps the
    jitted shard_map callable alive so repeat kernel() calls skip retracing.
    """
    if "runner" in _CACHE:
        return _CACHE["runner"]

    import jax
    import numpy as _np
    from jax.sharding import Mesh, PartitionSpec
    from jax.experimental.shard_map import shard_map
    from concourse import bass2jax, mybir as _mybir

    nc = build_nc()
    bass2jax.install_neuronx_cc_hook()

    partition_name = (nc.partition_id_tensor.name
                      if nc.partition_id_tensor else None)
    in_names, out_names, out_avals, zero_shapes = [], [], [], []
    for alloc in nc.m.functions[0].allocations:
        if not isinstance(alloc, _mybir.MemoryLocationSet):
            continue
        name = alloc.memorylocations[0].name
        if alloc.kind == "ExternalInput":
            if name != partition_name:
                in_names.append(name)
        elif alloc.kind == "ExternalOutput":
            shape = tuple(alloc.tensor_shape)
            dtype = _mybir.dt.np(alloc.dtype)
            out_names.append(name)
            out_avals.append(jax.core.ShapedArray(shape, dtype))
            zero_shapes.append((shape, dtype))
    n_params = len(in_names)
    n_outs = len(out_avals)
    all_in_names = list(in_names) + list(out_names)
    if partition_name is not None:
        all_in_names.append(partition_name)
    donate = tuple(range(n_params, n_params + n_outs))

    def _body(*args):
        operands = list(args)
        if partition_name is not None:
            operands.append(bass2jax.partition_id_tensor())
        outs = bass2jax._bass_exec_p.bind(
            *operands,
            out_avals=tuple(out_avals),
            in_names=tuple(all_in_names),
            out_names=tuple(out_names),
            lowering_input_output_aliases=(),
            sim_require_finite=True,
            sim_require_nnan=True,
            nc=nc,
        )
        return tuple(outs)

    devices = jax.devices()[:NCORES]
    mesh = Mesh(_np.asarray(devices), ("core",))
    in_specs = (PartitionSpec("core"),) * (n_params + n_outs)
    out_specs = (PartitionSpec("core"),) * n_outs
    sharded = jax.jit(
        shard_map(_body, mesh=mesh, in_specs=in_specs, out_specs=out_specs,
                  check_rep=False),
        donate_argnums=donate,
        keep_unused=True,
    )

    def run(in_maps):
        per_core = [[_np.asarray(m[name]) for name in in_names]
                    for m in in_maps]
        concat_in = [
            _np.concatenate([per_core[c][i] for c in range(NCORES)], axis=0)
            for i in range(n_params)
        ]
        concat_zeros = [
            _np.zeros((NCORES * s[0], *s[1:]), dt) for s, dt in zero_shapes
        ]
        out_arrs = sharded(*concat_in, *concat_zeros)
        return [
            {name: _np.asarray(out_arrs[i]).reshape(
                NCORES, *out_avals[i].shape)[c]
             for i, name in enumerate(out_names)}
            for c in range(NCORES)
        ]

    _CACHE["runner"] = run
    return run


def kernel(**inputs):
    run = _get_runner()
    in_maps = make_in_maps(**{k: np.asarray(v, dtype=np.float32)
                              for k, v in inputs.items()})
    return assemble_output(run(in_maps))


# revision 29
# speedup vs baseline: 1.3785x; 1.0550x over previous
"""Self-contained TRN2 Bass kernel for nn_EnhancedMultiheadAttention.

kernel(**inputs) takes the FULL unsharded inputs (x, Wq, bq, Wk, bk, Wv, bv,
Wo, bo as float32 numpy arrays), distributes the computation across 8
NeuronCores (tensor-parallel over heads: core c owns heads 2c, 2c+1), and
returns the full [2, 2048, 1024] float32 output.

v2 design notes:
- all matmul operands in bf16 (fast weight load, half DMA/SBUF traffic);
  accumulation stays fp32 in PSUM.
- QK^T packs both local heads into one PE pass via 64-row tile_position
  row-tiles running concurrently (heads stacked on partitions 0:64/64:128).
- V projection is computed directly in [token, dim] layout (lhsT=x chunk)
  so no PE transposes are needed; a ones-column rides along for the
  softmax denominator.
- output tokens are sharded interleaved per 1024-token block (core c owns
  tokens blk*1024 + c*128 ..+128) so one small bf16 AllToAll fires per
  block and overlaps the remaining attention; output projection runs in
  the tail, overlapping the last A2A.
- softmax denominator reciprocal uses reciprocal_approx_fast + a DRAM
  bounce row for the partition broadcast.
"""

import sys

for _p in ("/opt/trn_rl_repo", "/root/.axon_site/_ro/trn_rl_repo"):
    if _p not in sys.path:
        sys.path.append(_p)


import numpy as np

import concourse.bass as bass
import concourse.mybir as mybir
import concourse.tile as tile
import bass_rust

F32 = mybir.dt.float32
B16 = mybir.dt.bfloat16

B, L, D = 2, 2048, 1024
H, DH = 16, 64
NCORES = 8
T = B * L                  # 4096 flattened tokens
NKC = D // 128             # 8 contraction chunks of 128
CHUNK = 1024               # projection token-chunk width
IB = 1024                  # query block width
NI = L // IB               # 2 query blocks per batch
NJ = L // 128              # 16 key chunks of 128 per batch
HPC = H // NCORES          # 2 heads per core
NBLK = T // IB             # 4 token blocks (A2A granularity)
TPC_BLK = IB // NCORES     # 128 tokens per core per block
TC = NBLK * TPC_BLK        # 512 output tokens per core


def split_excess_waits(nc, max_waits=1):
    """walrus's setupSyncWait rejects instructions with more than one wait
    condition on this compiler version; hoist extras onto preceding NoOps."""
    n_split = 0
    for f in nc.m.functions:
        for b in f.blocks:
            new_list = None
            for inst in list(b.instructions):
                si = inst.sync_info
                if si is None or len(si.on_wait) <= max_waits:
                    continue
                waits = list(si.on_wait)
                keep = waits[-max_waits:]
                excess = waits[:-max_waits]
                nops = []
                for j, w in enumerate(excess):
                    nop = mybir.InstNoOp(
                        name=f"I-wsplit-{inst.name}-{j}", ins=[], outs=[],
                        engine=inst.engine,
                    )
                    nop.sync_info = bass_rust.SyncInfo(on_wait=[w], on_update=[])
                    nops.append(nop)
                inst.sync_info = bass_rust.SyncInfo(
                    on_wait=keep, on_update=list(si.on_update)
                )
                if new_list is None:
                    new_list = list(b.instructions)
                pos = new_list.index(inst)
                new_list[pos:pos] = nops
                n_split += 1
            if new_list is not None:
                b.instructions = new_list
    return n_split


def build_nc(proj_mm=None, attn_mm=None, out_mm=None):
    nc = bass.Bass("TRN2", target_bir_lowering=False, debug=False,
                   num_devices=NCORES)

    xT = nc.dram_tensor("xT", [D, T], B16, kind="ExternalInput").ap()
    wq = nc.dram_tensor("wq", [D, 128], B16, kind="ExternalInput").ap()
    wk = nc.dram_tensor("wk", [D, 128], B16, kind="ExternalInput").ap()
    wv = nc.dram_tensor("wv", [D, 128], B16, kind="ExternalInput").ap()
    bq = nc.dram_tensor("bq", [128, 1], F32, kind="ExternalInput").ap()
    bk = nc.dram_tensor("bk", [128, 1], F32, kind="ExternalInput").ap()
    bv = nc.dram_tensor("bv", [128, 1], F32, kind="ExternalInput").ap()
    wo = nc.dram_tensor("wo", [D, D], B16, kind="ExternalInput").ap()
    bo = nc.dram_tensor("bo", [1, D], F32, kind="ExternalInput").ap()
    cosT = nc.dram_tensor("cosT", [128, L], B16, kind="ExternalInput").ap()
    sinT = nc.dram_tensor("sinT", [128, L], B16, kind="ExternalInput").ap()
    out = nc.dram_tensor("out", [TC, D], F32, kind="ExternalOutput").ap()

    with tile.TileContext(nc) as tc:
        _build_body(nc, tc, xT, wq, wk, wv, bq, bk, bv, wo, bo, cosT, sinT,
                    out)

    split_excess_waits(nc)
    return nc


def _build_body(nc, tc, xT, wq, wk, wv, bq, bk, bv, wo, bo, cosT, sinT, out):
    from contextlib import ExitStack

    ctx = ExitStack()
    with ctx:
        # ---------------- persistent tensors ----------------
        persist = ctx.enter_context(tc.tile_pool(name="persist", bufs=1))
        wpool = ctx.enter_context(tc.tile_pool(name="wqkv", bufs=1))

        w_t = {}
        b_t = {}
        for name, wap, bap in (("q", wq, bq), ("k", wk, bk), ("v", wv, bv)):
            w_t[name] = wpool.tile([128, NKC, 128], B16, tag=f"w{name}",
                                   name=f"w{name}")
            nc.sync.dma_start(w_t[name][:],
                              wap.rearrange("(kc p) m -> p kc m", p=128))
            b_t[name] = wpool.tile([128, 1], F32, tag=f"b{name}",
                                   name=f"b{name}")
            nc.sync.dma_start(b_t[name][:], bap)

        cos_l = persist.tile([128, L], B16, tag="cos", name="cos")
        sin_l = persist.tile([128, L], B16, tag="sin", name="sin")
        nc.sync.dma_start(cos_l[:], cosT[:])
        nc.sync.dma_start(sin_l[:], sinT[:])

        ones_col = persist.tile([128, 1], F32, tag="ones", name="ones")
        nc.gpsimd.memset(ones_col[:], 1.0)

        ident = persist.tile([128, 128], B16, tag="ident", name="ident")
        from concourse.masks import make_identity
        make_identity(nc, ident[:])

        # per-batch Q/K (heads stacked on partitions) and V ([token, dim])
        qt_sb = [persist.tile([128, L], B16, tag=f"qt{b}", name=f"qt{b}")
                 for b in range(B)]
        kt_sb = [persist.tile([128, L], B16, tag=f"kt{b}", name=f"kt{b}")
                 for b in range(B)]
        v_sb = [[persist.tile([128, NJ, 66], B16, tag=f"v{b}{h}",
                              name=f"v{b}{h}")
                 for h in range(HPC)] for b in range(B)]
        for b in range(B):
            for h in range(HPC):
                nc.vector.tensor_copy(
                    v_sb[b][h][:, :, 64:65],
                    ones_col[:, :].to_broadcast((128, NJ, 1)))

        # context after A2A: [128 dims-of-kc, kc, blk, tok]
        opool = ctx.enter_context(tc.tile_pool(name="oproj", bufs=1))
        wo_sb = opool.tile([128, NKC, D], B16, tag="wo", name="wo")
        bo_sb = opool.tile([128, D], F32, tag="bo", name="bo")
        ctx_sb = opool.tile([128, NKC, NBLK, TPC_BLK], B16, tag="ctxsb",
                            name="ctxsb")

        # DRAM buffers for the two paired-block collectives (bf16)
        dram = ctx.enter_context(tc.tile_pool(name="dram", bufs=1,
                                              space="DRAM"))
        PAIRW = 2 * HPC * TPC_BLK
        a2a_in = [dram.tile([NCORES * DH, PAIRW], B16,
                            name=f"a2a_in{g}") for g in range(2)]
        a2a_out = [dram.tile([NCORES * DH, PAIRW], B16,
                             name=f"a2a_out{g}") for g in range(2)]

        xT3 = xT.rearrange("(kc p) t -> p kc t", p=128)

        # prefetch ALL x chunks up front (sync queue) so the inter-batch
        # transition never waits on HBM
        xpool = ctx.enter_context(tc.tile_pool(name="x", bufs=4))
        xt_c = []
        for ci in range(T // CHUNK):
            xt = xpool.tile([128, NKC, CHUNK], B16, tag="xchunk",
                            name=f"xchunk{ci}")
            for kc in range(NKC):
                nc.sync.dma_start(xt[:, kc, :], xT3[:, kc, bass.ts(ci, CHUNK)])
            xt_c.append(xt)
        # big output-projection weights load overlaps everything
        nc.sync.dma_start(wo_sb[:],
                          wo.rearrange("(kc p) n -> p kc n", p=128))
        nc.sync.dma_start(bo_sb[:], bo.to_broadcast((128, D)))

        # ---------------- phase A: projections + RoPE (all chunks) -------
        actx = ExitStack()
        tmp = actx.enter_context(tc.tile_pool(name="ptmp", bufs=3))
        ppsum = actx.enter_context(
            tc.tile_pool(name="ppsum", bufs=2, space="PSUM"))
        vtpsum = actx.enter_context(
            tc.tile_pool(name="vtpsum", bufs=2, space="PSUM"))

        for ci in range(T // CHUNK):
            b, half = divmod(ci, L // CHUNK)
            if True:
                lsl = bass.ts(half, CHUNK)         # slice into qt/kt/cos/sin
                xt = xt_c[ci]
                cos_t = cos_l[:, lsl]
                sin_t = sin_l[:, lsl]

                # Q/K/V projections: [dims, tokens]
                for name in ("q", "k", "v"):
                    ps = ppsum.tile([128, CHUNK], F32, tag="proj", name="proj")
                    for kc in range(NKC):
                        for nh in range(CHUNK // 512):
                            nc.tensor.matmul(
                                ps[:, bass.ts(nh, 512)],
                                w_t[name][:, kc, :],
                                xt[:, kc, bass.ts(nh, 512)],
                                start=(kc == 0), stop=(kc == NKC - 1),
                            )
                    if name == "v":
                        raw = tmp.tile([128, CHUNK], B16, tag="raw",
                                       name="raw")
                        nc.vector.tensor_scalar_add(raw[:], ps[:], b_t["v"][:])
                        # transpose [64, 128] blocks into [token, dim] via PE
                        for h in range(HPC):
                            o = h * DH
                            for sub in range(CHUNK // 128):
                                kb = half * (CHUNK // 128) + sub
                                tp = vtpsum.tile([128, DH], B16, tag="vt",
                                                 name="vt")
                                nc.tensor.transpose(
                                    tp[:],
                                    raw[bass.ds(o, DH), bass.ts(sub, 128)],
                                    ident[bass.ds(o, DH), bass.ds(o, DH)],
                                )
                                nc.vector.tensor_copy(
                                    v_sb[b][h][:, kb, 0:DH], tp[:])
                        continue
                    # Q/K: bias then RoPE
                    raw = tmp.tile([128, CHUNK], B16, tag="raw", name="raw")
                    nc.vector.tensor_scalar_add(raw[:], ps[:], b_t[name][:])
                    shifted = tmp.tile([128, CHUNK], B16, tag="shift",
                                       name="shift")
                    for h in range(HPC):
                        o = h * DH
                        nc.gpsimd.dma_start(shifted[o:o + 32, :],
                                            raw[o + 32:o + 64, :])
                        nc.gpsimd.dma_start(shifted[o + 32:o + 64, :],
                                            raw[o:o + 32, :])
                    t1 = tmp.tile([128, CHUNK], B16, tag="t1", name="t1")
                    nc.vector.tensor_mul(t1[:], raw[:], cos_t)
                    nc.vector.tensor_mul(shifted[:], shifted[:], sin_t)
                    dst = qt_sb[b] if name == "q" else kt_sb[b]
                    nc.vector.tensor_add(dst[:, lsl], t1[:], shifted[:])

        actx.close()

        # ---------------- phase B: attention (all blocks) ----------------
        bctx = ExitStack()
        ppool = bctx.enter_context(tc.tile_pool(name="pT", bufs=4))
        avspool = bctx.enter_context(tc.tile_pool(name="avs", bufs=2))
        rrpool = bctx.enter_context(tc.tile_pool(name="rr", bufs=2))
        rbpool = bctx.enter_context(tc.tile_pool(name="rb", bufs=2))
        cxpool = bctx.enter_context(tc.tile_pool(name="cx", bufs=2))
        stpsum = bctx.enter_context(
            tc.tile_pool(name="stpsum", bufs=2, space="PSUM"))
        avpsum = bctx.enter_context(
            tc.tile_pool(name="avpsum", bufs=2, space="PSUM"))
        ndram = bctx.enter_context(
            tc.tile_pool(name="ndram", bufs=2, space="DRAM"))

        for blk in range(NBLK):
            b, ib = divmod(blk, NI)
            if True:
                av = [avpsum.tile([128, IB], F32, tag="av", name="av")
                      for _ in range(HPC)]
                pt_prev = None
                for jc in range(NJ):
                    st = [stpsum.tile([128, IB], F32, tag="st", name="st")
                          for _ in range(HPC)]
                    ksl = bass.ds(jc * 128, 128)
                    # QK^T for both heads concurrently (64-row PE tiles)
                    for nh in range(IB // 512):
                        qsl = bass.ds(ib * IB + nh * 512, 512)
                        for h in range(HPC):
                            o = h * DH
                            nc.tensor.matmul(
                                st[h][:, bass.ts(nh, 512)],
                                kt_sb[b][o:o + DH, ksl],
                                qt_sb[b][o:o + DH, qsl],
                                start=True, stop=True,
                            )
                    pt = [ppool.tile([128, IB], B16, tag="pt", name="pt")
                          for _ in range(HPC)]
                    for h in range(HPC):
                        nc.scalar.activation(pt[h][:], st[h][:],
                                             mybir.ActivationFunctionType.Exp,
                                             scale=float(DH) ** -0.5)
                    if pt_prev is not None:
                        _emit_av(nc, av, v_sb[b], pt_prev, jc - 1)
                    pt_prev = pt
                _emit_av(nc, av, v_sb[b], pt_prev, NJ - 1)

                # normalize: den is row DH of av. Bounce it through DRAM
                # reshaped to [128, IB//128] so reciprocal runs on all 128
                # DVE lanes (the [1, IB] form is ~8 cyc/elem on one lane),
                # then bounce back and broadcast-read over DH partitions.
                for h in range(HPC):
                    avs = avspool.tile([DH + 1, IB], F32, tag="avs",
                                       name="avs")
                    nc.vector.tensor_copy(avs[:], av[h][0:DH + 1, :])
                    rr = rrpool.tile([DH + 1, IB], F32, tag="rr", name="rr")
                    nc.vector.reciprocal(rr[DH:DH + 1, :], avs[DH:DH + 1, :])
                    rrow = ndram.tile([1, IB], F32, tag="rrow", name="rrow")
                    nc.sync.dma_start(rrow[:], rr[DH:DH + 1, :])
                    rb = rbpool.tile([DH, IB], F32, tag="rb", name="rb")
                    nc.sync.dma_start(rb[:], rrow[0:1, :].to_broadcast((DH, IB)))
                    cx = cxpool.tile([DH, IB], B16, tag="cx", name="cx")
                    nc.vector.tensor_mul(cx[:], avs[0:DH, :], rb[:])
                    # stage into the pair A2A buffer: shard s gets this
                    # head's tokens s*128..+128 of this block
                    g = blk // 2
                    coff = (blk % 2) * HPC * TPC_BLK + h * TPC_BLK
                    for s in range(NCORES):
                        nc.sync.dma_start(
                            a2a_in[g][bass.ds(s * DH, DH),
                                      bass.ds(coff, TPC_BLK)],
                            cx[:, bass.ts(s, TPC_BLK)])

                if blk % 2 == 1:
                    g = blk // 2
                    nc.gpsimd.collective_compute(
                        "AllToAll",
                        mybir.AluOpType.bypass,
                        replica_groups=[list(range(NCORES))],
                        ins=[a2a_in[g][:]],
                        outs=[a2a_out[g][:]],
                    )
                    for pb in range(2):
                        rblk = 2 * g + pb
                        for h in range(HPC):
                            coff = pb * HPC * TPC_BLK + h * TPC_BLK
                            rsrc = a2a_out[g][:, bass.ds(coff, TPC_BLK)] \
                                .rearrange("(gg p) t -> p gg t", p=DH)
                            nc.gpsimd.dma_start(
                                ctx_sb[bass.ds(h * DH, DH), :, rblk, :], rsrc)

        bctx.close()

        # ---------------- phase C: output projection (tail) ----------------
        ostage = ctx.enter_context(tc.tile_pool(name="ostage", bufs=2))
        opsum = ctx.enter_context(tc.tile_pool(name="opsum", bufs=4,
                                               space="PSUM"))

        for blk in range(NBLK):
            pss = [opsum.tile([128, 512], F32, tag="ops", name=f"ops{blk}{nh}")
                   for nh in range(2)]
            for kc in range(NKC):
                for nh in range(2):
                    nc.tensor.matmul(
                        pss[nh][:],
                        ctx_sb[:, kc, blk, :],
                        wo_sb[:, kc, bass.ts(nh, 512)],
                        start=(kc == 0), stop=(kc == NKC - 1),
                    )
            for nh in range(2):
                ot = ostage.tile([128, 512], F32, tag="ot", name="ot")
                nc.vector.tensor_add(ot[:], pss[nh][:],
                                     bo_sb[:, bass.ts(nh, 512)])
                nc.sync.dma_start(
                    out[bass.ts(blk, TPC_BLK), bass.ts(nh, 512)], ot[:])


def _emit_av(nc, av, v_b, pt, jc):
    """AV accumulation for key chunk jc, both heads."""
    for h in range(HPC):
        for nh in range(IB // 512):
            nc.tensor.matmul(
                av[h][0:DH + 1, bass.ts(nh, 512)],
                v_b[h][:, jc, 0:DH + 1],
                pt[h][:, bass.ts(nh, 512)],
                start=(jc == 0), stop=(jc == NJ - 1),
            )


# ---------------- host-side sharding / unsharding ----------------

def _bf16(a):
    import ml_dtypes
    return np.ascontiguousarray(a).astype(ml_dtypes.bfloat16)


def rope_cos_sin_np(seq_len, d_head):
    inv_freq = 1.0 / (10000.0 ** (np.arange(0, d_head, 2, dtype=np.float32) / d_head))
    t = np.arange(seq_len, dtype=np.float32)
    freqs = np.einsum("i,j->ij", t, inv_freq).astype(np.float32)
    emb = np.concatenate((freqs, freqs), axis=-1)
    return np.cos(emb).astype(np.float32), np.sin(emb).astype(np.float32)


def make_in_maps(x, Wq, bq, Wk, bk, Wv, bv, Wo, bo):
    xT = _bf16(x.reshape(T, D).T)

    cos, sin = rope_cos_sin_np(L, DH)          # [L, 64]
    cosT = cos.T                               # [64, L]
    sinT = sin.T
    sgn = np.where(np.arange(DH) < DH // 2, -1.0, 1.0).astype(np.float32)
    sinT_signed = sinT * sgn[:, None]
    cosT_full = _bf16(np.tile(cosT, (HPC, 1)))      # [128, 2048]
    sinT_full = _bf16(np.tile(sinT_signed, (HPC, 1)))

    wo_full = _bf16(Wo)
    bo_row = np.ascontiguousarray(bo.reshape(1, D))

    in_maps = []
    for c in range(NCORES):
        sl = slice(c * 128, (c + 1) * 128)
        in_maps.append({
            "xT": xT,
            "wq": _bf16(Wq[:, sl]),
            "wk": _bf16(Wk[:, sl]),
            "wv": _bf16(Wv[:, sl]),
            "bq": np.ascontiguousarray(bq[sl].reshape(128, 1)),
            "bk": np.ascontiguousarray(bk[sl].reshape(128, 1)),
            "bv": np.ascontiguousarray(bv[sl].reshape(128, 1)),
            "wo": wo_full,
            "bo": bo_row,
            "cosT": cosT_full,
            "sinT": sinT_full,
        })
    return in_maps


def assemble_output(results):
    outs = np.stack([results[c]["out"] for c in range(NCORES)])  # [8, 512, D]
    full = outs.reshape(NCORES, NBLK, TPC_BLK, D).transpose(1, 0, 2, 3)
    return np.ascontiguousarray(full.reshape(B, L, D)).astype(np.float32)


_CACHE = {}


def _get_runner():
    """Build the Bass program and a cached jitted SPMD executor once.

    Mirrors bass2jax.run_bass_via_pjrt's multi-core path, but keeps the
    jitted shard_map callable alive so repeat kernel() calls skip retracing.
    """
    if "runner" in _CACHE:
        return _CACHE["runner"]

    import jax
    import numpy as _np
    from jax.sharding import Mesh, PartitionSpec
    from jax.experimental.shard_map import shard_map
    from concourse import bass2jax, mybir as _mybir

    nc = build_nc()
    bass2jax.install_neuronx_cc_hook()

    partition_name = (nc.partition_id_tensor.name
                      if nc.partition_id_tensor else None)
    in_names, out_names, out_avals, zero_shapes = [], [], [], []
    for alloc in nc.m.functions[0].allocations:
        if not isinstance(alloc, _mybir.MemoryLocationSet):
            continue
        name = alloc.memorylocations[0].name
        if alloc.kind == "ExternalInput":
            if name != partition_name:
                in_names.append(name)
        elif alloc.kind == "ExternalOutput":
            shape = tuple(alloc.tensor_shape)
            dtype = _mybir.dt.np(alloc.dtype)
            out_names.append(name)
            out_avals.append(jax.core.ShapedArray(shape, dtype))
            zero_shapes.append((shape, dtype))
    n_params = len(in_names)
    n_outs = len(out_avals)
    all_in_names = list(in_names) + list(out_names)
    if partition_name is not None:
        all_in_names.append(partition_name)
    donate = tuple(range(n_params, n_params + n_outs))

    def _body(*args):
        operands = list(args)
        if partition_name is not None:
            operands.append(bass2jax.partition_id_tensor())
        outs = bass2jax._bass_exec_p.bind(
            *operands,
            out_avals=tuple(out_avals),
            in_names=tuple(all_in_names),
            out_names=tuple(out_names),
            lowering_input_output_aliases=(),
            sim_require_finite=True,
            sim_require_nnan=True,
            nc=nc,
        )
        return tuple(outs)

    devices = jax.devices()[:NCORES]
    mesh = Mesh(_np.asarray(devices), ("core",))
    in_specs = (PartitionSpec("core"),) * (n_params + n_outs)
    out_specs = (PartitionSpec("core"),) * n_outs
    sharded = jax.jit(
        shard_map(_body, mesh=mesh, in_specs=in_specs, out_specs=out_specs,
                  check_rep=False),
        donate_argnums=donate,
        keep_unused=True,
    )

    def run(in_maps):
        per_core = [[_np.asarray(m[name]) for name in in_names]
                    for m in in_maps]
        concat_in = [
            _np.concatenate([per_core[c][i] for c in range(NCORES)], axis=0)
            for i in range(n_params)
        ]
        concat_zeros = [
            _np.zeros((NCORES * s[0], *s[1:]), dt) for s, dt in zero_shapes
        ]
        out_arrs = sharded(*concat_in, *concat_zeros)
        return [
            {name: _np.asarray(out_arrs[i]).reshape(
                NCORES, *out_avals[i].shape)[c]
             for i, name in enumerate(out_names)}
            for c in range(NCORES)
        ]

    _CACHE["runner"] = run
    return run


def kernel(**inputs):
    run = _get_runner()
    in_maps = make_in_maps(**{k: np.asarray(v, dtype=np.float32)
                              for k, v in inputs.items()})
    return assemble_output(run(in_maps))


# revision 30
# speedup vs baseline: 1.4100x; 1.0228x over previous
"""Self-contained TRN2 Bass kernel for nn_EnhancedMultiheadAttention.

kernel(**inputs) takes the FULL unsharded inputs (x, Wq, bq, Wk, bk, Wv, bv,
Wo, bo as float32 numpy arrays), distributes the computation across 8
NeuronCores (tensor-parallel over heads: core c owns heads 2c, 2c+1), and
returns the full [2, 2048, 1024] float32 output.

v2 design notes:
- all matmul operands in bf16 (fast weight load, half DMA/SBUF traffic);
  accumulation stays fp32 in PSUM.
- QK^T packs both local heads into one PE pass via 64-row tile_position
  row-tiles running concurrently (heads stacked on partitions 0:64/64:128).
- V projection is computed directly in [token, dim] layout (lhsT=x chunk)
  so no PE transposes are needed; a ones-column rides along for the
  softmax denominator.
- output tokens are sharded interleaved per 1024-token block (core c owns
  tokens blk*1024 + c*128 ..+128) so one small bf16 AllToAll fires per
  block and overlaps the remaining attention; output projection runs in
  the tail, overlapping the last A2A.
- softmax denominator reciprocal uses reciprocal_approx_fast + a DRAM
  bounce row for the partition broadcast.
"""

import sys

for _p in ("/opt/trn_rl_repo", "/root/.axon_site/_ro/trn_rl_repo"):
    if _p not in sys.path:
        sys.path.append(_p)


import numpy as np

import concourse.bass as bass
import concourse.mybir as mybir
import concourse.tile as tile
import bass_rust

F32 = mybir.dt.float32
B16 = mybir.dt.bfloat16

B, L, D = 2, 2048, 1024
H, DH = 16, 64
NCORES = 8
T = B * L                  # 4096 flattened tokens
NKC = D // 128             # 8 contraction chunks of 128
CHUNK = 1024               # projection token-chunk width
IB = 1024                  # query block width
NI = L // IB               # 2 query blocks per batch
NJ = L // 128              # 16 key chunks of 128 per batch
HPC = H // NCORES          # 2 heads per core
NBLK = T // IB             # 4 token blocks (A2A granularity)
TPC_BLK = IB // NCORES     # 128 tokens per core per block
TC = NBLK * TPC_BLK        # 512 output tokens per core


def split_excess_waits(nc, max_waits=1):
    """walrus's setupSyncWait rejects instructions with more than one wait
    condition on this compiler version; hoist extras onto preceding NoOps."""
    n_split = 0
    for f in nc.m.functions:
        for b in f.blocks:
            new_list = None
            for inst in list(b.instructions):
                si = inst.sync_info
                if si is None or len(si.on_wait) <= max_waits:
                    continue
                waits = list(si.on_wait)
                keep = waits[-max_waits:]
                excess = waits[:-max_waits]
                nops = []
                for j, w in enumerate(excess):
                    nop = mybir.InstNoOp(
                        name=f"I-wsplit-{inst.name}-{j}", ins=[], outs=[],
                        engine=inst.engine,
                    )
                    nop.sync_info = bass_rust.SyncInfo(on_wait=[w], on_update=[])
                    nops.append(nop)
                inst.sync_info = bass_rust.SyncInfo(
                    on_wait=keep, on_update=list(si.on_update)
                )
                if new_list is None:
                    new_list = list(b.instructions)
                pos = new_list.index(inst)
                new_list[pos:pos] = nops
                n_split += 1
            if new_list is not None:
                b.instructions = new_list
    return n_split


def build_nc(proj_mm=None, attn_mm=None, out_mm=None):
    nc = bass.Bass("TRN2", target_bir_lowering=False, debug=False,
                   num_devices=NCORES)

    xT = nc.dram_tensor("xT", [D, T], B16, kind="ExternalInput").ap()
    wq = nc.dram_tensor("wq", [D, 128], B16, kind="ExternalInput").ap()
    wk = nc.dram_tensor("wk", [D, 128], B16, kind="ExternalInput").ap()
    wv = nc.dram_tensor("wv", [D, 128], B16, kind="ExternalInput").ap()
    bq = nc.dram_tensor("bq", [128, 1], F32, kind="ExternalInput").ap()
    bk = nc.dram_tensor("bk", [128, 1], F32, kind="ExternalInput").ap()
    bv = nc.dram_tensor("bv", [128, 1], F32, kind="ExternalInput").ap()
    wo = nc.dram_tensor("wo", [D, D], B16, kind="ExternalInput").ap()
    bo = nc.dram_tensor("bo", [1, D], F32, kind="ExternalInput").ap()
    cosT = nc.dram_tensor("cosT", [128, L], B16, kind="ExternalInput").ap()
    sinT = nc.dram_tensor("sinT", [128, L], B16, kind="ExternalInput").ap()
    out = nc.dram_tensor("out", [TC, D], F32, kind="ExternalOutput").ap()

    with tile.TileContext(nc) as tc:
        _build_body(nc, tc, xT, wq, wk, wv, bq, bk, bv, wo, bo, cosT, sinT,
                    out)

    split_excess_waits(nc)
    return nc


def _build_body(nc, tc, xT, wq, wk, wv, bq, bk, bv, wo, bo, cosT, sinT, out):
    from contextlib import ExitStack

    ctx = ExitStack()
    with ctx:
        # ---------------- persistent tensors ----------------
        persist = ctx.enter_context(tc.tile_pool(name="persist", bufs=1))
        wpool = ctx.enter_context(tc.tile_pool(name="wqkv", bufs=1))

        w_t = {}
        b_t = {}
        for name, wap, bap in (("q", wq, bq), ("k", wk, bk), ("v", wv, bv)):
            w_t[name] = wpool.tile([128, NKC, 128], B16, tag=f"w{name}",
                                   name=f"w{name}")
            nc.sync.dma_start(w_t[name][:],
                              wap.rearrange("(kc p) m -> p kc m", p=128))
            b_t[name] = wpool.tile([128, 1], F32, tag=f"b{name}",
                                   name=f"b{name}")
            nc.sync.dma_start(b_t[name][:], bap)

        cos_l = persist.tile([128, L], B16, tag="cos", name="cos")
        sin_l = persist.tile([128, L], B16, tag="sin", name="sin")
        nc.sync.dma_start(cos_l[:], cosT[:])
        nc.sync.dma_start(sin_l[:], sinT[:])

        ones_col = persist.tile([128, 1], F32, tag="ones", name="ones")
        nc.gpsimd.memset(ones_col[:], 1.0)

        ident = persist.tile([128, 128], B16, tag="ident", name="ident")
        from concourse.masks import make_identity
        make_identity(nc, ident[:])

        # per-batch Q/K (heads stacked on partitions) and V ([token, dim])
        qt_sb = [persist.tile([128, L], B16, tag=f"qt{b}", name=f"qt{b}")
                 for b in range(B)]
        kt_sb = [persist.tile([128, L], B16, tag=f"kt{b}", name=f"kt{b}")
                 for b in range(B)]
        v_sb = [[persist.tile([128, NJ, 66], B16, tag=f"v{b}{h}",
                              name=f"v{b}{h}")
                 for h in range(HPC)] for b in range(B)]
        for b in range(B):
            for h in range(HPC):
                nc.vector.tensor_copy(
                    v_sb[b][h][:, :, 64:65],
                    ones_col[:, :].to_broadcast((128, NJ, 1)))

        # context after A2A: [128 dims-of-kc, kc, blk, tok]
        opool = ctx.enter_context(tc.tile_pool(name="oproj", bufs=1))
        wo_sb = opool.tile([128, NKC, D], B16, tag="wo", name="wo")
        bo_sb = opool.tile([128, D], F32, tag="bo", name="bo")
        ctx_sb = opool.tile([128, NKC, NBLK, TPC_BLK], B16, tag="ctxsb",
                            name="ctxsb")

        # DRAM buffers for the collectives (bf16): blocks 0+1 share one
        # A2A (fires at the 50% mark), blocks 2 and 3 go alone so only the
        # small final A2A is exposed in the tail
        dram = ctx.enter_context(tc.tile_pool(name="dram", bufs=1,
                                              space="DRAM"))
        BLKW = HPC * TPC_BLK
        GRP = [(0, 1), (2,), (3,)]           # blocks per collective
        GOF = {0: (0, 0), 1: (0, 1), 2: (1, 0), 3: (2, 0)}  # blk -> (g, slot)
        a2a_in = [dram.tile([NCORES * DH, len(gr) * BLKW], B16,
                            name=f"a2a_in{g}") for g, gr in enumerate(GRP)]
        a2a_out = [dram.tile([NCORES * DH, len(gr) * BLKW], B16,
                             name=f"a2a_out{g}") for g, gr in enumerate(GRP)]

        xT3 = xT.rearrange("(kc p) t -> p kc t", p=128)

        # prefetch ALL x chunks up front (sync queue) so the inter-batch
        # transition never waits on HBM
        xpool = ctx.enter_context(tc.tile_pool(name="x", bufs=4))
        xt_c = []
        for ci in range(T // CHUNK):
            xt = xpool.tile([128, NKC, CHUNK], B16, tag="xchunk",
                            name=f"xchunk{ci}")
            for kc in range(NKC):
                nc.sync.dma_start(xt[:, kc, :], xT3[:, kc, bass.ts(ci, CHUNK)])
            xt_c.append(xt)
        # big output-projection weights load overlaps everything
        nc.sync.dma_start(wo_sb[:],
                          wo.rearrange("(kc p) n -> p kc n", p=128))
        nc.sync.dma_start(bo_sb[:], bo.to_broadcast((128, D)))

        # ---------------- phase A: projections + RoPE (all chunks) -------
        actx = ExitStack()
        tmp = actx.enter_context(tc.tile_pool(name="ptmp", bufs=3))
        ppsum = actx.enter_context(
            tc.tile_pool(name="ppsum", bufs=2, space="PSUM"))
        vtpsum = actx.enter_context(
            tc.tile_pool(name="vtpsum", bufs=2, space="PSUM"))

        for ci in range(T // CHUNK):
            b, half = divmod(ci, L // CHUNK)
            if True:
                lsl = bass.ts(half, CHUNK)         # slice into qt/kt/cos/sin
                xt = xt_c[ci]
                cos_t = cos_l[:, lsl]
                sin_t = sin_l[:, lsl]

                # Q/K/V projections: [dims, tokens]
                for name in ("q", "k", "v"):
                    ps = ppsum.tile([128, CHUNK], F32, tag="proj", name="proj")
                    for kc in range(NKC):
                        for nh in range(CHUNK // 512):
                            nc.tensor.matmul(
                                ps[:, bass.ts(nh, 512)],
                                w_t[name][:, kc, :],
                                xt[:, kc, bass.ts(nh, 512)],
                                start=(kc == 0), stop=(kc == NKC - 1),
                            )
                    if name == "v":
                        raw = tmp.tile([128, CHUNK], B16, tag="raw",
                                       name="raw")
                        nc.vector.tensor_scalar_add(raw[:], ps[:], b_t["v"][:])
                        # transpose [64, 128] blocks into [token, dim] via PE
                        for h in range(HPC):
                            o = h * DH
                            for sub in range(CHUNK // 128):
                                kb = half * (CHUNK // 128) + sub
                                tp = vtpsum.tile([128, DH], B16, tag="vt",
                                                 name="vt")
                                nc.tensor.transpose(
                                    tp[:],
                                    raw[bass.ds(o, DH), bass.ts(sub, 128)],
                                    ident[bass.ds(o, DH), bass.ds(o, DH)],
                                )
                                nc.vector.tensor_copy(
                                    v_sb[b][h][:, kb, 0:DH], tp[:])
                        continue
                    # Q/K: bias then RoPE
                    raw = tmp.tile([128, CHUNK], B16, tag="raw", name="raw")
                    nc.vector.tensor_scalar_add(raw[:], ps[:], b_t[name][:])
                    shifted = tmp.tile([128, CHUNK], B16, tag="shift",
                                       name="shift")
                    for h in range(HPC):
                        o = h * DH
                        nc.gpsimd.dma_start(shifted[o:o + 32, :],
                                            raw[o + 32:o + 64, :])
                        nc.gpsimd.dma_start(shifted[o + 32:o + 64, :],
                                            raw[o:o + 32, :])
                    t1 = tmp.tile([128, CHUNK], B16, tag="t1", name="t1")
                    nc.vector.tensor_mul(t1[:], raw[:], cos_t)
                    nc.vector.tensor_mul(shifted[:], shifted[:], sin_t)
                    dst = qt_sb[b] if name == "q" else kt_sb[b]
                    nc.vector.tensor_add(dst[:, lsl], t1[:], shifted[:])

        actx.close()

        # ---------------- phase B: attention (all blocks) ----------------
        bctx = ExitStack()
        ppool = bctx.enter_context(tc.tile_pool(name="pT", bufs=4))
        avspool = bctx.enter_context(tc.tile_pool(name="avs", bufs=2))
        rrpool = bctx.enter_context(tc.tile_pool(name="rr", bufs=2))
        rbpool = bctx.enter_context(tc.tile_pool(name="rb", bufs=2))
        cxpool = bctx.enter_context(tc.tile_pool(name="cx", bufs=2))
        stpsum = bctx.enter_context(
            tc.tile_pool(name="stpsum", bufs=2, space="PSUM"))
        avpsum = bctx.enter_context(
            tc.tile_pool(name="avpsum", bufs=2, space="PSUM"))
        ndram = bctx.enter_context(
            tc.tile_pool(name="ndram", bufs=2, space="DRAM"))

        for blk in range(NBLK):
            b, ib = divmod(blk, NI)
            if True:
                av = [avpsum.tile([128, IB], F32, tag="av", name="av")
                      for _ in range(HPC)]
                pt_prev = None
                for jc in range(NJ):
                    st = [stpsum.tile([128, IB], F32, tag="st", name="st")
                          for _ in range(HPC)]
                    ksl = bass.ds(jc * 128, 128)
                    # QK^T for both heads concurrently (64-row PE tiles)
                    for nh in range(IB // 512):
                        qsl = bass.ds(ib * IB + nh * 512, 512)
                        for h in range(HPC):
                            o = h * DH
                            nc.tensor.matmul(
                                st[h][:, bass.ts(nh, 512)],
                                kt_sb[b][o:o + DH, ksl],
                                qt_sb[b][o:o + DH, qsl],
                                start=True, stop=True,
                            )
                    pt = [ppool.tile([128, IB], B16, tag="pt", name="pt")
                          for _ in range(HPC)]
                    for h in range(HPC):
                        nc.scalar.activation(pt[h][:], st[h][:],
                                             mybir.ActivationFunctionType.Exp,
                                             scale=float(DH) ** -0.5)
                    if pt_prev is not None:
                        _emit_av(nc, av, v_sb[b], pt_prev, jc - 1)
                    pt_prev = pt
                _emit_av(nc, av, v_sb[b], pt_prev, NJ - 1)

                # normalize: den is row DH of av. Bounce it through DRAM
                # reshaped to [128, IB//128] so reciprocal runs on all 128
                # DVE lanes (the [1, IB] form is ~8 cyc/elem on one lane),
                # then bounce back and broadcast-read over DH partitions.
                for h in range(HPC):
                    avs = avspool.tile([DH + 1, IB], F32, tag="avs",
                                       name="avs")
                    nc.vector.tensor_copy(avs[:], av[h][0:DH + 1, :])
                    rr = rrpool.tile([DH + 1, IB], F32, tag="rr", name="rr")
                    nc.vector.reciprocal(rr[DH:DH + 1, :], avs[DH:DH + 1, :])
                    rrow = ndram.tile([1, IB], F32, tag="rrow", name="rrow")
                    nc.sync.dma_start(rrow[:], rr[DH:DH + 1, :])
                    rb = rbpool.tile([DH, IB], F32, tag="rb", name="rb")
                    nc.sync.dma_start(rb[:], rrow[0:1, :].to_broadcast((DH, IB)))
                    cx = cxpool.tile([DH, IB], B16, tag="cx", name="cx")
                    nc.vector.tensor_mul(cx[:], avs[0:DH, :], rb[:])
                    # stage into the A2A group buffer: shard s gets this
                    # head's tokens s*128..+128 of this block
                    g, slot = GOF[blk]
                    coff = slot * BLKW + h * TPC_BLK
                    for s in range(NCORES):
                        nc.sync.dma_start(
                            a2a_in[g][bass.ds(s * DH, DH),
                                      bass.ds(coff, TPC_BLK)],
                            cx[:, bass.ts(s, TPC_BLK)])

                g, slot = GOF[blk]
                if blk == GRP[g][-1]:
                    nc.gpsimd.collective_compute(
                        "AllToAll",
                        mybir.AluOpType.bypass,
                        replica_groups=[list(range(NCORES))],
                        ins=[a2a_in[g][:]],
                        outs=[a2a_out[g][:]],
                    )
                    for pb, rblk in enumerate(GRP[g]):
                        for h in range(HPC):
                            coff = pb * BLKW + h * TPC_BLK
                            rsrc = a2a_out[g][:, bass.ds(coff, TPC_BLK)] \
                                .rearrange("(gg p) t -> p gg t", p=DH)
                            nc.gpsimd.dma_start(
                                ctx_sb[bass.ds(h * DH, DH), :, rblk, :], rsrc)

        bctx.close()

        # ---------------- phase C: output projection (tail) ----------------
        ostage = ctx.enter_context(tc.tile_pool(name="ostage", bufs=2))
        opsum = ctx.enter_context(tc.tile_pool(name="opsum", bufs=4,
                                               space="PSUM"))

        for blk in range(NBLK):
            pss = [opsum.tile([128, 512], F32, tag="ops", name=f"ops{blk}{nh}")
                   for nh in range(2)]
            for kc in range(NKC):
                for nh in range(2):
                    nc.tensor.matmul(
                        pss[nh][:],
                        ctx_sb[:, kc, blk, :],
                        wo_sb[:, kc, bass.ts(nh, 512)],
                        start=(kc == 0), stop=(kc == NKC - 1),
                    )
            for nh in range(2):
                ot = ostage.tile([128, 512], F32, tag="ot", name="ot")
                nc.vector.tensor_add(ot[:], pss[nh][:],
                                     bo_sb[:, bass.ts(nh, 512)])
                nc.sync.dma_start(
                    out[bass.ts(blk, TPC_BLK), bass.ts(nh, 512)], ot[:])


def _emit_av(nc, av, v_b, pt, jc):
    """AV accumulation for key chunk jc, both heads."""
    for h in range(HPC):
        for nh in range(IB // 512):
            nc.tensor.matmul(
                av[h][0:DH + 1, bass.ts(nh, 512)],
                v_b[h][:, jc, 0:DH + 1],
                pt[h][:, bass.ts(nh, 512)],
                start=(jc == 0), stop=(jc == NJ - 1),
            )


# ---------------- host-side sharding / unsharding ----------------

def _bf16(a):
    import ml_dtypes
    return np.ascontiguousarray(a).astype(ml_dtypes.bfloat16)


def rope_cos_sin_np(seq_len, d_head):
    inv_freq = 1.0 / (10000.0 ** (np.arange(0, d_head, 2, dtype=np.float32) / d_head))
    t = np.arange(seq_len, dtype=np.float32)
    freqs = np.einsum("i,j->ij", t, inv_freq).astype(np.float32)
    emb = np.concatenate((freqs, freqs), axis=-1)
    return np.cos(emb).astype(np.float32), np.sin(emb).astype(np.float32)


def make_in_maps(x, Wq, bq, Wk, bk, Wv, bv, Wo, bo):
    xT = _bf16(x.reshape(T, D).T)

    cos, sin = rope_cos_sin_np(L, DH)          # [L, 64]
    cosT = cos.T                               # [64, L]
    sinT = sin.T
    sgn = np.where(np.arange(DH) < DH // 2, -1.0, 1.0).astype(np.float32)
    sinT_signed = sinT * sgn[:, None]
    cosT_full = _bf16(np.tile(cosT, (HPC, 1)))      # [128, 2048]
    sinT_full = _bf16(np.tile(sinT_signed, (HPC, 1)))

    wo_full = _bf16(Wo)
    bo_row = np.ascontiguousarray(bo.reshape(1, D))

    in_maps = []
    for c in range(NCORES):
        sl = slice(c * 128, (c + 1) * 128)
        in_maps.append({
            "xT": xT,
            "wq": _bf16(Wq[:, sl]),
            "wk": _bf16(Wk[:, sl]),
            "wv": _bf16(Wv[:, sl]),
            "bq": np.ascontiguousarray(bq[sl].reshape(128, 1)),
            "bk": np.ascontiguousarray(bk[sl].reshape(128, 1)),
            "bv": np.ascontiguousarray(bv[sl].reshape(128, 1)),
            "wo": wo_full,
            "bo": bo_row,
            "cosT": cosT_full,
            "sinT": sinT_full,
        })
    return in_maps


def assemble_output(results):
    outs = np.stack([results[c]["out"] for c in range(NCORES)])  # [8, 512, D]
    full = outs.reshape(NCORES, NBLK, TPC_BLK, D).transpose(1, 0, 2, 3)
    return np.ascontiguousarray(full.reshape(B, L, D)).astype(np.float32)


_CACHE = {}


def _get_runner():
    """Build the Bass program and a cached jitted SPMD executor once.

    Mirrors bass2jax.run_bass_via_pjrt's multi-core path, but keeps the
    jitted shard_map callable alive so repeat kernel() calls skip retracing.
    """
    if "runner" in _CACHE:
        return _CACHE["runner"]

    import jax
    import numpy as _np
    from jax.sharding import Mesh, PartitionSpec
    from jax.experimental.shard_map import shard_map
    from concourse import bass2jax, mybir as _mybir

    nc = build_nc()
    bass2jax.install_neuronx_cc_hook()

    partition_name = (nc.partition_id_tensor.name
                      if nc.partition_id_tensor else None)
    in_names, out_names, out_avals, zero_shapes = [], [], [], []
    for alloc in nc.m.functions[0].allocations:
        if not isinstance(alloc, _mybir.MemoryLocationSet):
            continue
        name = alloc.memorylocations[0].name
        if alloc.kind == "ExternalInput":
            if name != partition_name:
                in_names.append(name)
        elif alloc.kind == "ExternalOutput":
            shape = tuple(alloc.tensor_shape)
            dtype = _mybir.dt.np(alloc.dtype)
            out_names.append(name)
            out_avals.append(jax.core.ShapedArray(shape, dtype))
            zero_shapes.append((shape, dtype))
    n_params = len(in_names)
    n_outs = len(out_avals)
    all_in_names = list(in_names) + list(out_names)
    if partition_name is not None:
        all_in_names.append(partition_name)
    donate = tuple(range(n_params, n_params + n_outs))

    def _body(*args):
        operands = list(args)
        if partition_name is not None:
            operands.append(bass2jax.partition_id_tensor())
        outs = bass2jax._bass_exec_p.bind(
            *operands,
            out_avals=tuple(out_avals),
            in_names=tuple(all_in_names),
            out_names=tuple(out_names),
            lowering_input_output_aliases=(),
            sim_require_finite=True,
            sim_require_nnan=True,
            nc=nc,
        )
        return tuple(outs)

    devices = jax.devices()[:NCORES]
    mesh = Mesh(_np.asarray(devices), ("core",))
    in_specs = (PartitionSpec("core"),) * (n_params + n_outs)
    out_specs = (PartitionSpec("core"),) * n_outs
    sharded = jax.jit(
        shard_map(_body, mesh=mesh, in_specs=in_specs, out_specs=out_specs,
                  check_rep=False),
        donate_argnums=donate,
        keep_unused=True,
    )

    def run(in_maps):
        per_core = [[_np.asarray(m[name]) for name in in_names]
                    for m in in_maps]
        concat_in = [
            _np.concatenate([per_core[c][i] for c in range(NCORES)], axis=0)
            for i in range(n_params)
        ]
        concat_zeros = [
            _np.zeros((NCORES * s[0], *s[1:]), dt) for s, dt in zero_shapes
        ]
        out_arrs = sharded(*concat_in, *concat_zeros)
        return [
            {name: _np.asarray(out_arrs[i]).reshape(
                NCORES, *out_avals[i].shape)[c]
             for i, name in enumerate(out_names)}
            for c in range(NCORES)
        ]

    _CACHE["runner"] = run
    return run


def kernel(**inputs):
    run = _get_runner()
    in_maps = make_in_maps(**{k: np.asarray(v, dtype=np.float32)
                              for k, v in inputs.items()})
    return assemble_output(run(in_maps))
